# revision 28
# baseline (speedup 1.0000x reference)
# Trainium2 Bass kernel for nn_LiteMultiscaleAttention (8-core data-parallel over batch).
#
# Per core (one batch):
#   P1 qkv = Wqkv @ x (bf16) -> fp8 padded copy (agg rhs) + bf16 q-rows (stage B)
#   P2 qkvT = xT @ WqkvT (bf16, k/v cols only) streamed into stage-A vk matmuls
#      (kills the 256 PE transposes for clean heads)
#   P3 agg = fused dw5x5+grouped-pw as 25 block-diag taps in fp8, with
#      DoubleRow perf mode pairing vertical taps (2 taps/instr, row stride 80)
#   P4 stage-A for agg heads: fp8 PE transposes + vk matmuls
#   P5 stage-B: 2 matmuls per (tg, nt): block-diag num [128x128], rank-1 den
#   P6 proj + BN (bf16)
import sys
import os
import numpy as np

sys.path.insert(0, '/opt/trn_rl_repo')

import ml_dtypes
import concourse.bass as bass
import concourse.mybir as mybir
import concourse.tile as tile
from concourse import bacc
from concourse.bass_utils import run_bass_kernel_spmd
from concourse.masks import make_identity

BF16 = mybir.dt.bfloat16
F32 = mybir.dt.float32
FP8 = mybir.dt.float8e4
DR = mybir.MatmulPerfMode.DoubleRow

B, CIN, H, W = 8, 512, 64, 64
S = H * W                 # 4096
C3 = 1536
NCT = 12                  # channel tiles of qkv/agg
NKT = 4                   # k-tiles of Cin
NNT = 8                   # 512-wide spatial chunks (8 image rows each)
EPS = 1e-15
BN_EPS = 1e-5
RS = 80                   # padded row stride (so DoubleRow pair step % 16 == 0)
NR = 68                   # padded rows
W2_SCALE = 2.0 ** 14      # host scale on fused agg weights (fp8 range)
AST_SCALE = 2.0 ** 6      # storage scale of agg outputs in fp8
AST_EVAC = AST_SCALE / W2_SCALE

# tap slots: pairs (dy,dy+1) same dx -> DoubleRow; dy=4 row as singles
PAIRS = [((0, dx), (1, dx)) for dx in range(5)] + [((2, dx), (3, dx)) for dx in range(5)]
SINGLES = [(4, dx) for dx in range(5)]
SLOT_TAPS = []
for (a, b_) in PAIRS:
    SLOT_TAPS += [a, b_]
SLOT_TAPS += SINGLES      # 25 slots: [0..19] pair slots, [20..24] singles

_CACHED = {}


def _qkv_unit_role(c):
    """channel c -> (head, role, lane) with role 0=q 1=k 2=v."""
    return c // 48, (c % 48) // 16, c % 16


def build_program(dbg=False):
    nc = bacc.Bacc('TRN2', target_bir_lowering=False, debug=False)

    x8_d = nc.dram_tensor('x8', [128, NKT, S], BF16, kind='ExternalInput')
    wq_d = nc.dram_tensor('wq', [128, NKT, C3], BF16, kind='ExternalInput')
    wqT_d = nc.dram_tensor('wqT', [128, NKT, 1024], BF16, kind='ExternalInput')
    w2c_d = nc.dram_tensor('w2c', [128, 25, NCT, 32], FP8, kind='ExternalInput')
    wp_d = nc.dram_tensor('wp', [128, 8, 512], BF16, kind='ExternalInput')
    bnb_d = nc.dram_tensor('bnb', [128, 4], F32, kind='ExternalInput')
    obd32_d = nc.dram_tensor('obd32', [128, 32], BF16, kind='ExternalInput')
    obd128_d = nc.dram_tensor('obd128', [128, 128], BF16, kind='ExternalInput')
    y_d = nc.dram_tensor('y_b', [512, S], F32, kind='ExternalOutput')
    if dbg:
        dbg_qkv = nc.dram_tensor('dbg_qkv', [128, NCT, NR, RS], FP8, kind='ExternalOutput')
        dbg_qstc = nc.dram_tensor('dbg_qstc', [128, 4, S], BF16, kind='ExternalOutput')
        dbg_qsta = nc.dram_tensor('dbg_qsta', [128, 4, S], FP8, kind='ExternalOutput')
        dbg_bdn = nc.dram_tensor('dbg_bdn', [128, 8, 128], BF16, kind='ExternalOutput')
        dbg_bdd = nc.dram_tensor('dbg_bdd', [128, 8, 128], BF16, kind='ExternalOutput')
        dbg_den = nc.dram_tensor('dbg_den', [128, 8, 1], F32, kind='ExternalOutput')
        dbg_attn = nc.dram_tensor('dbg_attn', [128, 8, S], BF16, kind='ExternalOutput')
        dbg_kh = nc.dram_tensor('dbg_kh', [128, 4, S], FP8, kind='ExternalOutput')
        dbg_vh = nc.dram_tensor('dbg_vh', [128, 4, S], FP8, kind='ExternalOutput')

    with tile.TileContext(nc) as tc:
        from contextlib import ExitStack
        ctx = ExitStack()
        with ctx:
            stat = ctx.enter_context(tc.tile_pool(name='stat', bufs=1))
            id8 = stat.tile([128, 128], FP8)
            make_identity(nc, id8[:])
            ones_b = stat.tile([128, 1], BF16)
            nc.gpsimd.memset(ones_b[:], 1.0)
            ones_8 = stat.tile([128, 1], FP8)
            nc.gpsimd.memset(ones_8[:], 1.0)
            ones512 = stat.tile([1, 512], BF16)
            nc.gpsimd.memset(ones512[:], 1.0)
            epsw = stat.tile([1, 128], BF16)
            nc.gpsimd.memset(epsw[:], EPS)
            obd32 = stat.tile([128, 32], BF16)
            nc.sync.dma_start(obd32[:], obd32_d.ap())
            obd128 = stat.tile([128, 128], BF16)
            nc.sync.dma_start(obd128[:], obd128_d.ap())

            bdn = stat.tile([128, 8, 128], BF16)
            nc.gpsimd.memset(bdn[:], 0.0)
            bdd = stat.tile([128, 8, 128], BF16)
            nc.gpsimd.memset(bdd[:], 0.0)
            den_col = stat.tile([128, 8, 1], F32)

            # long-lived data tiles
            qstcp = ctx.enter_context(tc.tile_pool(name='qstcp', bufs=1))
            qstc = qstcp.tile([128, 4, S], BF16)      # clean q (heads 0-31)
            qstap = ctx.enter_context(tc.tile_pool(name='qstap', bufs=1))
            qsta = qstap.tile([128, 4, S], FP8)       # agg q (heads 32-63)

            es_qkv = ExitStack()
            qkvp = es_qkv.enter_context(tc.tile_pool(name='qkvp', bufs=1))
            qkv = qkvp.tile([128, NCT, NR, RS], FP8)
            for _ct in range(NCT):
                nc.gpsimd.memset(qkv[:, _ct, 0:2, 0:68], 0.0)
                nc.gpsimd.memset(qkv[:, _ct, 66:68, 0:68], 0.0)
                nc.gpsimd.memset(qkv[:, _ct, 2:66, 0:2], 0.0)
                nc.gpsimd.memset(qkv[:, _ct, 2:66, 66:68], 0.0)

            # ---------------- P0/P1: qkv normal ----------------
            es1 = ExitStack()
            w1 = es1.enter_context(tc.tile_pool(name='w1', bufs=1))
            wq = w1.tile([128, NKT, C3], BF16)
            wqT = w1.tile([128, NKT, 1024], BF16)
            x8 = w1.tile([128, NKT, S], BF16)
            for kt in range(NKT):
                nc.sync.dma_start(wq[:, kt, :], wq_d.ap()[:, kt, :])
                nc.sync.dma_start(x8[:, kt, :], x8_d.ap()[:, kt, :])
            nc.sync.dma_start(wqT[:], wqT_d.ap())

            qunits = {ct: [] for ct in range(NCT)}
            for u in range(96):
                c = 16 * u
                h, role, _ = _qkv_unit_role(c)
                if role == 0:
                    qunits[c // 128].append(((c % 128) // 16, h))

            es_p1 = ExitStack()
            stgp = es_p1.enter_context(tc.tile_pool(name='stgp', bufs=3))
            psum1 = es_p1.enter_context(tc.tile_pool(name='psum1', bufs=1, space='PSUM'))
            ev = [nc.vector, nc.scalar]
            evi = 0
            for ct in range(NCT):
                for half in range(2):
                    pss = {}
                    for kt in range(NKT):
                        for q4 in range(4):
                            nt = 4 * half + q4
                            if kt == 0:
                                pss[nt] = psum1.tile([128, 512], F32, tag=f'ps{q4}', bufs=2, name=f'p1ps{q4}')
                            nc.tensor.matmul(
                                pss[nt][:], wq[:, kt, 128 * ct:128 * (ct + 1)],
                                x8[:, kt, 512 * nt:512 * (nt + 1)],
                                start=(kt == 0), stop=(kt == NKT - 1))
                    for q4 in range(4):
                        nt = 4 * half + q4
                        ps = pss[nt]
                        dst = qkv[:, ct, 8 * nt + 2:8 * nt + 10, 2:66]
                        stg = stgp.tile([128, 512], BF16, tag='stg', name='stg')
                        if (nt % 2) == 0:
                            nc.vector.tensor_copy(dst, ps[:])
                            nc.scalar.activation(stg[:], ps[:],
                                                 mybir.ActivationFunctionType.Copy)
                        else:
                            nc.scalar.activation(dst, ps[:], mybir.ActivationFunctionType.Copy)
                            nc.vector.tensor_copy(stg[:], ps[:])
                        for (bi, h) in qunits[ct]:
                            dq = qstc[16 * (h % 8):16 * (h % 8) + 16, h // 8,
                                      512 * nt:512 * (nt + 1)]
                            nc.gpsimd.dma_start(dq, stg[16 * bi:16 * bi + 16, :])

            # ---------------- P2: qkvT + stage A for clean heads ----------------
            es_p1.close()
            es_p2 = ExitStack()
            psum2 = es_p2.enter_context(tc.tile_pool(name='psum2', bufs=1, space='PSUM'))
            psA = {}
            for tg in range(4):
                psA[tg] = psum2.tile([128, 136], F32, tag=f'psA{tg}', bufs=1, name=f'psA{tg}')
            with tc.tile_pool(name='tkv', bufs=2) as tkvp:
                tvs = [tkvp.tile([128, 4, 132], BF16, tag='tv', name=f'tv{i}') for i in range(2)]
                for i in range(2):
                    nc.gpsimd.memset(tvs[i][:, :, 128:129], 1.0)
                for c in range(32):
                    psT = psum2.tile([128, 1024], F32, tag='psT', bufs=2)
                    xa = x8[:, :, 128 * c:128 * (c + 1)]
                    for kt in range(NKT):
                        nc.tensor.matmul(psT[:, 0:512], xa[:, kt, :], wqT[:, kt, 0:512],
                                         start=(kt == 0), stop=(kt == NKT - 1))
                    for kt in range(NKT):
                        nc.tensor.matmul(psT[:, 512:1024], xa[:, kt, :], wqT[:, kt, 512:1024],
                                         start=(kt == 0), stop=(kt == NKT - 1))
                    tk = tkvp.tile([128, 512], BF16, tag='tk')
                    tv = tvs[c % 2]
                    nc.vector.tensor_scalar_max(tk[:], psT[:, 0:512], 0.0)
                    nc.scalar.activation(tv[:, :, 0:128], psT[:, 512:1024],
                                         mybir.ActivationFunctionType.Copy)
                    for tg in range(4):
                        sl = slice(128 * tg, 128 * (tg + 1))
                        nc.tensor.matmul(psA[tg][:, 0:129], tk[:, sl], tv[:, tg, 0:129],
                                         start=(c == 0), stop=(c == 31))

            def evac_stage_a(tg, ps):
                for j in range(4):
                    r = slice(32 * j, 32 * (j + 1))
                    nc.vector.scalar_tensor_tensor(
                        bdn[r, tg, 32 * j:32 * j + 32], ps[r, 32 * j:32 * j + 32], 1.0,
                        obd32[r, :], mybir.AluOpType.mult, mybir.AluOpType.mult)
                nc.vector.tensor_copy(den_col[:, tg, :], ps[:, 128:129])
                nc.vector.tensor_scalar_mul(bdd[:, tg, :], obd128[:], den_col[:, tg, :])

            for tg in range(4):
                evac_stage_a(tg, psA[tg])
            es_p2.close()
            es1.close()   # frees x8/wq/wqT

            # ---------------- P3: agg (fp8 DoubleRow taps) ----------------
            es3 = ExitStack()
            w2cp = es3.enter_context(tc.tile_pool(name='w2cp', bufs=1))
            w2c = w2cp.tile([128, 25, NCT, 32], FP8)
            nc.sync.dma_start(w2c[:], w2c_d.ap())
            holdp = es3.enter_context(tc.tile_pool(name='holdp', bufs=2))
            khold = {}
            vhold = {}
            trp = es3.enter_context(tc.tile_pool(name='trp', bufs=4))
            vTts = [trp.tile([128, 132], FP8, tag='vTt', name=f'vTt{i}') for i in range(4)]
            for i in range(4):
                nc.gpsimd.memset(vTts[i][:, 128:129], 1.0)
            lhsp = es3.enter_context(tc.tile_pool(name='lhsp', bufs=1))
            lts = [lhsp.tile([128, 25, 128], FP8, name=f'lt{i}') for i in range(2)]
            for i in range(2):
                nc.gpsimd.memset(lts[i][:], 0.0)
            astp = es3.enter_context(tc.tile_pool(name='astp', bufs=2))
            es_p3 = ExitStack()
            psum3 = es_p3.enter_context(tc.tile_pool(name='psum3', bufs=1, space='PSUM'))

            def pair_rhs(ct, nt, dy, dx, ddy):
                a = qkv[:, ct, 8 * nt + dy:8 * nt + dy + 8, dx:dx + 64]
                newdims = [list(a.ap[0])] + [[RS * ddy, 2]] + [list(d) for d in a.ap[1:]]
                return bass.AP(a.tensor, a.offset, newdims)

            for ct in range(NCT):
                lt = lts[ct % 2]
                for blk in range(4):
                    nc.sync.dma_start(lt[32 * blk:32 * blk + 32, :, 32 * blk:32 * blk + 32],
                                      w2c[32 * blk:32 * blk + 32, :, ct, :])
                ast = astp.tile([128, S], FP8, tag='ast')
                for nt in range(NNT):
                    ps = psum3.tile([128, 512], F32, tag='ps', bufs=2, name='p3ps')
                    for p, ((dy0, dx0), (dy1, dx1)) in enumerate(PAIRS):
                        nc.tensor.matmul(
                            ps[:], lt[:, 2 * p:2 * p + 2, :],
                            pair_rhs(ct, nt, dy0, dx0, dy1 - dy0),
                            start=(p == 0), stop=False, perf_mode=DR)
                    for si, (dy, dx) in enumerate(SINGLES):
                        nc.tensor.matmul(
                            ps[:], lt[:, 20 + si, :],
                            qkv[:, ct, 8 * nt + dy:8 * nt + dy + 8, dx:dx + 64],
                            start=False, stop=(si == 4))
                    dst = ast[:, 512 * nt:512 * (nt + 1)]
                    if nt % 2 == 0:
                        nc.vector.tensor_scalar_mul(dst, ps[:], AST_EVAC)
                    else:
                        nc.scalar.activation(dst, ps[:],
                                             mybir.ActivationFunctionType.Copy,
                                             scale=AST_EVAC)
                # route 16-row blocks to agg-head destinations
                for bi in range(8):
                    c = 128 * ct + 16 * bi
                    h = 32 + c // 48
                    r = c % 48
                    tg = h // 8
                    hl = h % 8
                    if tg not in khold:
                        khold[tg] = holdp.tile([128, S], FP8, tag='kh', name=f'kh{tg}')
                        vhold[tg] = holdp.tile([128, S], FP8, tag='vh', name=f'vh{tg}')
                    src = ast[16 * bi:16 * bi + 16, :]
                    if r < 16:
                        nc.sync.dma_start(qsta[16 * hl:16 * hl + 16, tg - 4, :], src)
                    elif r < 32:
                        nc.sync.dma_start(khold[tg][16 * hl:16 * hl + 16, :], src)
                    else:
                        nc.sync.dma_start(vhold[tg][16 * hl:16 * hl + 16, :], src)
                if ct % 3 == 2:
                    tg = 4 + ct // 3
                    psA2 = psum3.tile([128, 136], F32, tag='psA2', name='psA2')
                    for st in range(32):
                        sl = slice(128 * st, 128 * (st + 1))
                        psTk = psum3.tile([128, 128, 2], FP8, tag='trk', bufs=2, name='psTk')
                        nc.tensor.transpose(psTk[:, :, 0], khold[tg][:, sl], id8[:])
                        kTt = trp.tile([128, 128], FP8, tag='kTt', name='kTt')
                        if st % 2 == 0:
                            nc.vector.tensor_scalar_max(kTt[:], psTk[:, :, 0], 0.0)
                        else:
                            nc.scalar.activation(kTt[:], psTk[:, :, 0],
                                                 mybir.ActivationFunctionType.Relu)
                        psTv = psum3.tile([128, 128, 2], FP8, tag='trv', bufs=2, name='psTv')
                        nc.tensor.transpose(psTv[:, :, 0], vhold[tg][:, sl], id8[:])
                        vTt = vTts[st % 4]
                        if st % 2 == 1:
                            nc.vector.tensor_copy(vTt[:, 0:128], psTv[:, :, 0])
                        else:
                            nc.scalar.activation(vTt[:, 0:128], psTv[:, :, 0],
                                                 mybir.ActivationFunctionType.Copy)
                        nc.tensor.matmul(psA2[:, 0:129], kTt[:], vTt[:, 0:129],
                                         start=(st == 0), stop=(st == 31))
                    evac_stage_a(tg, psA2)
                    del khold[tg], vhold[tg]

            if dbg:
                nc.sync.dma_start(dbg_qkv.ap(), qkv[:])
                nc.sync.dma_start(dbg_qstc.ap(), qstc[:])
                nc.sync.dma_start(dbg_qsta.ap(), qsta[:])

            es_p3.close()
            es3.close()   # frees w2c, lt, ast, holds
            es_qkv.close()  # frees padded qkv

            # ---------------- P5+P6 fused, nt-outer ----------------
            wpp = ctx.enter_context(tc.tile_pool(name='wpp', bufs=1))
            wp = wpp.tile([128, 8, 512], BF16)
            nc.sync.dma_start(wp[:], wp_d.ap())
            bnb = wpp.tile([128, 4], F32)
            nc.sync.dma_start(bnb[:], bnb_d.ap())

            psum5 = ctx.enter_context(tc.tile_pool(name='psum5', bufs=1, space='PSUM'))
            qbp = ctx.enter_context(tc.tile_pool(name='qbp', bufs=1))
            attnp = ctx.enter_context(tc.tile_pool(name='attnp', bufs=2))
            drp = ctx.enter_context(tc.tile_pool(name='drp', bufs=2))

            qbs = {}
            for tg in range(8):
                qb = qbp.tile([128, S], BF16, name=f'qb{tg}')
                if tg < 4:
                    nc.vector.tensor_scalar_max(qb[:, 0:2048], qstc[:, tg, 0:2048], 0.0)
                    nc.scalar.activation(qb[:, 2048:4096], qstc[:, tg, 2048:4096],
                                         mybir.ActivationFunctionType.Relu)
                else:
                    nc.vector.tensor_scalar_max(qb[:, 0:2048], qsta[:, tg - 4, 0:2048], 0.0)
                    nc.scalar.activation(qb[:, 2048:4096], qsta[:, tg - 4, 2048:4096],
                                         mybir.ActivationFunctionType.Relu)
                qbs[tg] = qb

            with tc.tile_pool(name='ysp', bufs=3) as ysp:
                for nt in range(NNT):
                    sl = slice(512 * nt, 512 * (nt + 1))
                    attn_nt = attnp.tile([128, 8, 512], BF16, tag='attn_nt', name='attn_nt')
                    for tg in range(8):
                        qb = qbs[tg]
                        csc = 1.0 if tg < 4 else 1.0 / AST_SCALE
                        psN = psum5.tile([128, 512], F32, tag='psN', bufs=2, name='psN')
                        nc.tensor.matmul(psN[:], bdn[:, tg, :], qb[:, sl],
                                         start=True, stop=True)
                        psD = psum5.tile([128, 512], F32, tag='psD', bufs=2, name='psD')
                        nc.tensor.matmul(psD[:], bdd[:, tg, :], qb[:, sl],
                                         start=True, stop=True)
                        drt = drp.tile([128, 512], F32, tag='drt', name='drt')
                        nc.scalar.activation(drt[:], psD[:],
                                             mybir.ActivationFunctionType.Copy, bias=EPS)
                        nc.vector.reciprocal_approx_fast(drt[:], drt[:])
                        nc.vector.scalar_tensor_tensor(
                            attn_nt[:, tg, :], psN[:], csc, drt[:],
                            mybir.AluOpType.mult, mybir.AluOpType.mult)
                    for mt in range(4):
                        ps = psum5.tile([128, 512], F32, tag='psy', bufs=2, name='psy')
                        for kt in range(8):
                            nc.tensor.matmul(ps[:], wp[:, kt, 128 * mt:128 * (mt + 1)],
                                             attn_nt[:, kt, :],
                                             start=(kt == 0), stop=(kt == 7))
                        ys = ysp.tile([128, 512], F32, tag='ys')
                        if mt % 2 == 0:
                            nc.vector.tensor_scalar_add(ys[:], ps[:], bnb[:, mt:mt + 1])
                        else:
                            nc.scalar.activation(ys[:], ps[:],
                                                 mybir.ActivationFunctionType.Identity,
                                                 bias=bnb[:, mt:mt + 1])
                        nc.sync.dma_start(
                            y_d.ap()[128 * mt:128 * (mt + 1), 512 * nt:512 * (nt + 1)], ys[:])

    nc.compile()
    return nc


def host_weights(w_qkv, w_dw, w_pw, w_proj, bn_gamma, bn_beta, bn_mean, bn_var):
    wq = w_qkv[:, :, 0, 0].astype(np.float32)       # [1536, 512]
    wdw = w_dw[:, 0].reshape(1536, 25).astype(np.float32)
    wpw = w_pw[:, :, 0, 0].astype(np.float32)       # [1536, 32]

    # normal qkv weights, lhsT per k-tile
    wq_dev = np.ascontiguousarray(
        wq.T.reshape(NKT, 128, C3).transpose(1, 0, 2)).astype(ml_dtypes.bfloat16)

    # transposed-qkv rhs: k/v channels only, ordered [k: h0..h31 | v: h0..h31]
    kv_cols = np.empty(1024, np.int64)
    for h in range(32):
        kv_cols[16 * h:16 * h + 16] = 48 * h + 16 + np.arange(16)
        kv_cols[512 + 16 * h:512 + 16 * h + 16] = 48 * h + 32 + np.arange(16)
    wqT_dev = np.ascontiguousarray(
        wq[kv_cols].T.reshape(NKT, 128, 1024).transpose(1, 0, 2)).astype(ml_dtypes.bfloat16)

    # fused agg weights (compact block-diag), slot-ordered, scaled 2^14 -> fp8
    A = wdw.reshape(12, 4, 32, 25)
    Bm = wpw.reshape(12, 4, 32, 32)
    W2c = A[:, :, :, :, None] * Bm.transpose(0, 1, 3, 2)[:, :, :, None, :]
    W2c = W2c.transpose(1, 2, 3, 0, 4).reshape(128, 25, 12, 32)   # [row, tap, ct, o]
    slot_idx = [5 * dy + dx for (dy, dx) in SLOT_TAPS]
    W2s = np.clip(W2c[:, slot_idx] * W2_SCALE, -240.0, 240.0)
    w2c_dev = np.ascontiguousarray(W2s).astype(ml_dtypes.float8_e4m3)

    inv = bn_gamma / np.sqrt(bn_var + BN_EPS)
    wp_f = (w_proj[:, :, 0, 0] * inv[:, None]).T    # [1024, 512] lhsT
    bnb = (bn_beta - bn_mean * inv).astype(np.float32)
    wp_dev = np.ascontiguousarray(
        wp_f.reshape(8, 128, 512).transpose(1, 0, 2)).astype(ml_dtypes.bfloat16)
    bnb_dev = np.ascontiguousarray(bnb.reshape(4, 128).T).astype(np.float32)

    obd32 = np.zeros((128, 32), np.float32)
    for r in range(128):
        a = (r % 32) // 16
        obd32[r, 16 * a:16 * a + 16] = 1.0
    obd128 = np.zeros((128, 128), np.float32)
    for r in range(128):
        a = r // 16
        obd128[r, 16 * a:16 * a + 16] = 1.0
    return {'wq': wq_dev, 'wqT': wqT_dev, 'w2c': w2c_dev, 'wp': wp_dev,
            'bnb': bnb_dev,
            'obd32': obd32.astype(ml_dtypes.bfloat16),
            'obd128': obd128.astype(ml_dtypes.bfloat16)}


def host_x(xb):
    """[512, 4096] f32 -> [128, 4, 4096] bf16 tile layout."""
    return np.ascontiguousarray(
        xb.reshape(NKT, 128, S).transpose(1, 0, 2)).astype(ml_dtypes.bfloat16)


def kernel(x, w_qkv, w_dw, w_pw, w_proj, bn_gamma, bn_beta, bn_mean, bn_var):
    x = np.asarray(x, dtype=np.float32)
    wdev = host_weights(
        np.asarray(w_qkv, np.float32), np.asarray(w_dw, np.float32),
        np.asarray(w_pw, np.float32), np.asarray(w_proj, np.float32),
        np.asarray(bn_gamma, np.float32), np.asarray(bn_beta, np.float32),
        np.asarray(bn_mean, np.float32), np.asarray(bn_var, np.float32))

    if 'nc' not in _CACHED:
        _CACHED['nc'] = build_program()
    nc = _CACHED['nc']

    in_maps = []
    for b in range(B):
        in_maps.append({'x8': host_x(x[b].reshape(CIN, S)), **wdev})
    res = run_bass_kernel_spmd(nc, in_maps, list(range(B)))
    y = np.stack([res.results[b]['y_b'].reshape(512, H, W) for b in range(B)])
    return y.astype(np.float32)


# revision 29
# speedup vs baseline: 1.0994x; 1.0994x over previous
# Trainium2 Bass kernel for nn_LiteMultiscaleAttention (8-core data-parallel over batch).
#
# Per core (one batch):
#   P1 qkv = Wqkv @ x (bf16) -> fp8 padded copy (agg rhs) + bf16 q-rows (stage B)
#   P2 qkvT = xT @ WqkvT (bf16, k/v cols only) streamed into stage-A vk matmuls
#      (kills the 256 PE transposes for clean heads)
#   P3 agg = fused dw5x5+grouped-pw as 25 block-diag taps in fp8, with
#      DoubleRow perf mode pairing vertical taps (2 taps/instr, row stride 80)
#   P4 stage-A for agg heads: fp8 PE transposes + vk matmuls
#   P5 stage-B: 2 matmuls per (tg, nt): block-diag num [128x128], rank-1 den
#   P6 proj + BN (bf16)
import sys
import os
import numpy as np

sys.path.insert(0, '/opt/trn_rl_repo')

import ml_dtypes
import concourse.bass as bass
import concourse.mybir as mybir
import concourse.tile as tile
from concourse import bacc
from concourse.bass_utils import run_bass_kernel_spmd
from concourse.masks import make_identity

BF16 = mybir.dt.bfloat16
F32 = mybir.dt.float32
FP8 = mybir.dt.float8e4
DR = mybir.MatmulPerfMode.DoubleRow

B, CIN, H, W = 8, 512, 64, 64
S = H * W                 # 4096
C3 = 1536
NCT = 12                  # channel tiles of qkv/agg
NKT = 4                   # k-tiles of Cin
NNT = 8                   # 512-wide spatial chunks (8 image rows each)
EPS = 1e-15
BN_EPS = 1e-5
RS = 80                   # padded row stride (so DoubleRow pair step % 16 == 0)
NR = 68                   # padded rows
W2_SCALE = 2.0 ** 14      # host scale on fused agg weights (fp8 range)
AST_SCALE = 2.0 ** 6      # storage scale of agg outputs in fp8
AST_EVAC = AST_SCALE / W2_SCALE

# tap slots: pairs (dy,dy+1) same dx -> DoubleRow; dy=4 row as singles
PAIRS = [((0, dx), (1, dx)) for dx in range(5)] + [((2, dx), (3, dx)) for dx in range(5)]
SINGLES = [(4, dx) for dx in range(5)]
SLOT_TAPS = []
for (a, b_) in PAIRS:
    SLOT_TAPS += [a, b_]
SLOT_TAPS += SINGLES      # 25 slots: [0..19] pair slots, [20..24] singles

_CACHED = {}


def _qkv_unit_role(c):
    """channel c -> (head, role, lane) with role 0=q 1=k 2=v."""
    return c // 48, (c % 48) // 16, c % 16


def build_program(dbg=False):
    nc = bacc.Bacc('TRN2', target_bir_lowering=False, debug=False)

    x8_d = nc.dram_tensor('x8', [128, NKT, S], BF16, kind='ExternalInput')
    wq_d = nc.dram_tensor('wq', [128, NKT, C3], BF16, kind='ExternalInput')
    wqT_d = nc.dram_tensor('wqT', [128, NKT, 1024], BF16, kind='ExternalInput')
    wqq_d = nc.dram_tensor('wqq', [128, NKT, 512], BF16, kind='ExternalInput')
    w2c_d = nc.dram_tensor('w2c', [128, 25, NCT, 32], FP8, kind='ExternalInput')
    wp_d = nc.dram_tensor('wp', [128, 8, 512], BF16, kind='ExternalInput')
    bnb_d = nc.dram_tensor('bnb', [128, 4], F32, kind='ExternalInput')
    obd32_d = nc.dram_tensor('obd32', [128, 32], BF16, kind='ExternalInput')
    obd128_d = nc.dram_tensor('obd128', [128, 128], BF16, kind='ExternalInput')
    y_d = nc.dram_tensor('y_b', [512, S], F32, kind='ExternalOutput')
    if dbg:
        dbg_qkv = nc.dram_tensor('dbg_qkv', [128, NCT, NR, RS], FP8, kind='ExternalOutput')
        dbg_qstc = nc.dram_tensor('dbg_qstc', [128, 4, S], BF16, kind='ExternalOutput')
        dbg_qsta = nc.dram_tensor('dbg_qsta', [128, 4, S], FP8, kind='ExternalOutput')
        dbg_bdn = nc.dram_tensor('dbg_bdn', [128, 8, 128], BF16, kind='ExternalOutput')
        dbg_bdd = nc.dram_tensor('dbg_bdd', [128, 8, 128], BF16, kind='ExternalOutput')
        dbg_den = nc.dram_tensor('dbg_den', [128, 8, 1], F32, kind='ExternalOutput')
        dbg_attn = nc.dram_tensor('dbg_attn', [128, 8, S], BF16, kind='ExternalOutput')
        dbg_kh = nc.dram_tensor('dbg_kh', [128, 4, S], FP8, kind='ExternalOutput')
        dbg_vh = nc.dram_tensor('dbg_vh', [128, 4, S], FP8, kind='ExternalOutput')

    with tile.TileContext(nc) as tc:
        from contextlib import ExitStack
        ctx = ExitStack()
        with ctx:
            stat = ctx.enter_context(tc.tile_pool(name='stat', bufs=1))
            id8 = stat.tile([128, 128], FP8)
            make_identity(nc, id8[:])
            ones_b = stat.tile([128, 1], BF16)
            nc.gpsimd.memset(ones_b[:], 1.0)
            ones_8 = stat.tile([128, 1], FP8)
            nc.gpsimd.memset(ones_8[:], 1.0)
            ones512 = stat.tile([1, 512], BF16)
            nc.gpsimd.memset(ones512[:], 1.0)
            epsw = stat.tile([1, 128], BF16)
            nc.gpsimd.memset(epsw[:], EPS)
            obd32 = stat.tile([128, 32], BF16)
            nc.sync.dma_start(obd32[:], obd32_d.ap())
            obd128 = stat.tile([128, 128], BF16)
            nc.sync.dma_start(obd128[:], obd128_d.ap())

            bdn = stat.tile([128, 8, 128], BF16)
            nc.gpsimd.memset(bdn[:], 0.0)
            bdd = stat.tile([128, 8, 128], BF16)
            nc.gpsimd.memset(bdd[:], 0.0)
            den_col = stat.tile([128, 8, 1], F32)

            # long-lived data tiles
            qstcp = ctx.enter_context(tc.tile_pool(name='qstcp', bufs=1))
            qstc = qstcp.tile([128, 4, S], BF16)      # clean q (heads 0-31)
            qstap = ctx.enter_context(tc.tile_pool(name='qstap', bufs=1))
            qsta = qstap.tile([128, 4, S], FP8)       # agg q (heads 32-63)

            es_qkv = ExitStack()
            qkvp = es_qkv.enter_context(tc.tile_pool(name='qkvp', bufs=1))
            qkv = qkvp.tile([128, NCT, NR, RS], FP8)
            for _ct in range(NCT):
                nc.gpsimd.memset(qkv[:, _ct, 0:2, 0:68], 0.0)
                nc.gpsimd.memset(qkv[:, _ct, 66:68, 0:68], 0.0)
                nc.gpsimd.memset(qkv[:, _ct, 2:66, 0:2], 0.0)
                nc.gpsimd.memset(qkv[:, _ct, 2:66, 66:68], 0.0)

            # ---------------- P0/P1: qkv normal ----------------
            es1 = ExitStack()
            w1 = es1.enter_context(tc.tile_pool(name='w1', bufs=1))
            wq = w1.tile([128, NKT, C3], BF16)
            wqT = w1.tile([128, NKT, 1024], BF16)
            wqq = w1.tile([128, NKT, 512], BF16)
            x8 = w1.tile([128, NKT, S], BF16)
            for kt in range(NKT):
                nc.sync.dma_start(wq[:, kt, :], wq_d.ap()[:, kt, :])
                nc.sync.dma_start(x8[:, kt, :], x8_d.ap()[:, kt, :])
            nc.sync.dma_start(wqT[:], wqT_d.ap())
            nc.sync.dma_start(wqq[:], wqq_d.ap())

            es_p1 = ExitStack()
            psum1 = es_p1.enter_context(tc.tile_pool(name='psum1', bufs=1, space='PSUM'))
            ev = [nc.vector, nc.scalar]
            evi = 0
            for ct in range(NCT):
                for half in range(2):
                    pss = {}
                    for kt in range(NKT):
                        for q4 in range(4):
                            nt = 4 * half + q4
                            if kt == 0:
                                pss[nt] = psum1.tile([128, 512], F32, tag=f'ps{q4}', bufs=2, name=f'p1ps{q4}')
                            nc.tensor.matmul(
                                pss[nt][:], wq[:, kt, 128 * ct:128 * (ct + 1)],
                                x8[:, kt, 512 * nt:512 * (nt + 1)],
                                start=(kt == 0), stop=(kt == NKT - 1))
                    for q4 in range(4):
                        nt = 4 * half + q4
                        ps = pss[nt]
                        dst = qkv[:, ct, 8 * nt + 2:8 * nt + 10, 2:66]
                        if (nt % 2) == 0:
                            nc.vector.tensor_copy(dst, ps[:])
                        else:
                            nc.scalar.activation(dst, ps[:], mybir.ActivationFunctionType.Copy)

            # P1b: q-channels again, head-major -> clean q-stack (full-tile evacs)
            for qt in range(4):
                for half in range(2):
                    pss = {}
                    for kt in range(NKT):
                        for q4 in range(4):
                            nt = 4 * half + q4
                            if kt == 0:
                                pss[nt] = psum1.tile([128, 512], F32, tag=f'ps{q4}', bufs=2, name=f'pbps{q4}')
                            nc.tensor.matmul(
                                pss[nt][:], wqq[:, kt, 128 * qt:128 * (qt + 1)],
                                x8[:, kt, 512 * nt:512 * (nt + 1)],
                                start=(kt == 0), stop=(kt == NKT - 1))
                    for q4 in range(4):
                        nt = 4 * half + q4
                        dq = qstc[:, qt, 512 * nt:512 * (nt + 1)]
                        if (nt % 2) == 0:
                            nc.vector.tensor_copy(dq, pss[nt][:])
                        else:
                            nc.scalar.activation(dq, pss[nt][:], mybir.ActivationFunctionType.Copy)

            # ---------------- P2: qkvT + stage A for clean heads ----------------
            es_p1.close()
            es_p2 = ExitStack()
            psum2 = es_p2.enter_context(tc.tile_pool(name='psum2', bufs=1, space='PSUM'))
            psA = {}
            for tg in range(4):
                psA[tg] = psum2.tile([128, 136], F32, tag=f'psA{tg}', bufs=1, name=f'psA{tg}')
            with tc.tile_pool(name='tkv', bufs=2) as tkvp:
                tvs = [tkvp.tile([128, 4, 132], BF16, tag='tv', name=f'tv{i}') for i in range(2)]
                for i in range(2):
                    nc.gpsimd.memset(tvs[i][:, :, 128:129], 1.0)
                for c in range(32):
                    psT = psum2.tile([128, 1024], F32, tag='psT', bufs=2)
                    xa = x8[:, :, 128 * c:128 * (c + 1)]
                    for kt in range(NKT):
                        nc.tensor.matmul(psT[:, 0:512], xa[:, kt, :], wqT[:, kt, 0:512],
                                         start=(kt == 0), stop=(kt == NKT - 1))
                    for kt in range(NKT):
                        nc.tensor.matmul(psT[:, 512:1024], xa[:, kt, :], wqT[:, kt, 512:1024],
                                         start=(kt == 0), stop=(kt == NKT - 1))
                    tk = tkvp.tile([128, 512], BF16, tag='tk')
                    tv = tvs[c % 2]
                    nc.vector.tensor_scalar_max(tk[:], psT[:, 0:512], 0.0)
                    nc.scalar.activation(tv[:, :, 0:128], psT[:, 512:1024],
                                         mybir.ActivationFunctionType.Copy)
                    for tg in range(4):
                        sl = slice(128 * tg, 128 * (tg + 1))
                        nc.tensor.matmul(psA[tg][:, 0:129], tk[:, sl], tv[:, tg, 0:129],
                                         start=(c == 0), stop=(c == 31))

            def evac_stage_a(tg, ps):
                for j in range(4):
                    r = slice(32 * j, 32 * (j + 1))
                    nc.vector.scalar_tensor_tensor(
                        bdn[r, tg, 32 * j:32 * j + 32], ps[r, 32 * j:32 * j + 32], 1.0,
                        obd32[r, :], mybir.AluOpType.mult, mybir.AluOpType.mult)
                nc.vector.tensor_copy(den_col[:, tg, :], ps[:, 128:129])
                nc.vector.tensor_scalar_mul(bdd[:, tg, :], obd128[:], den_col[:, tg, :])

            for tg in range(4):
                evac_stage_a(tg, psA[tg])
            es_p2.close()
            es1.close()   # frees x8/wq/wqT

            # ---------------- P3: agg (fp8 DoubleRow taps) ----------------
            es3 = ExitStack()
            w2cp = es3.enter_context(tc.tile_pool(name='w2cp', bufs=1))
            w2c = w2cp.tile([128, 25, NCT, 32], FP8)
            nc.sync.dma_start(w2c[:], w2c_d.ap())
            holdp = es3.enter_context(tc.tile_pool(name='holdp', bufs=2))
            khold = {}
            vhold = {}
            trp = es3.enter_context(tc.tile_pool(name='trp', bufs=4))
            vTts = [trp.tile([128, 132], FP8, tag='vTt', name=f'vTt{i}') for i in range(4)]
            for i in range(4):
                nc.gpsimd.memset(vTts[i][:, 128:129], 1.0)
            lhsp = es3.enter_context(tc.tile_pool(name='lhsp', bufs=1))
            lts = [lhsp.tile([128, 25, 128], FP8, name=f'lt{i}') for i in range(2)]
            for i in range(2):
                nc.gpsimd.memset(lts[i][:], 0.0)
            astp = es3.enter_context(tc.tile_pool(name='astp', bufs=2))
            es_p3 = ExitStack()
            psum3 = es_p3.enter_context(tc.tile_pool(name='psum3', bufs=1, space='PSUM'))

            def pair_rhs(ct, nt, dy, dx, ddy):
                a = qkv[:, ct, 8 * nt + dy:8 * nt + dy + 8, dx:dx + 64]
                newdims = [list(a.ap[0])] + [[RS * ddy, 2]] + [list(d) for d in a.ap[1:]]
                return bass.AP(a.tensor, a.offset, newdims)

            for ct in range(NCT):
                lt = lts[ct % 2]
                for blk in range(4):
                    nc.sync.dma_start(lt[32 * blk:32 * blk + 32, :, 32 * blk:32 * blk + 32],
                                      w2c[32 * blk:32 * blk + 32, :, ct, :])
                ast = astp.tile([128, S], FP8, tag='ast')
                for nt in range(NNT):
                    ps = psum3.tile([128, 512], F32, tag='ps', bufs=2, name='p3ps')
                    for p, ((dy0, dx0), (dy1, dx1)) in enumerate(PAIRS):
                        nc.tensor.matmul(
                            ps[:], lt[:, 2 * p:2 * p + 2, :],
                            pair_rhs(ct, nt, dy0, dx0, dy1 - dy0),
                            start=(p == 0), stop=False, perf_mode=DR)
                    for si, (dy, dx) in enumerate(SINGLES):
                        nc.tensor.matmul(
                            ps[:], lt[:, 20 + si, :],
                            qkv[:, ct, 8 * nt + dy:8 * nt + dy + 8, dx:dx + 64],
                            start=False, stop=(si == 4))
                    dst = ast[:, 512 * nt:512 * (nt + 1)]
                    if nt % 2 == 0:
                        nc.vector.tensor_scalar_mul(dst, ps[:], AST_EVAC)
                    else:
                        nc.scalar.activation(dst, ps[:],
                                             mybir.ActivationFunctionType.Copy,
                                             scale=AST_EVAC)
                # route 16-row blocks to agg-head destinations
                for bi in range(8):
                    c = 128 * ct + 16 * bi
                    h = 32 + c // 48
                    r = c % 48
                    tg = h // 8
                    hl = h % 8
                    if tg not in khold:
                        khold[tg] = holdp.tile([128, S], FP8, tag='kh', name=f'kh{tg}')
                        vhold[tg] = holdp.tile([128, S], FP8, tag='vh', name=f'vh{tg}')
                    src = ast[16 * bi:16 * bi + 16, :]
                    if r < 16:
                        nc.sync.dma_start(qsta[16 * hl:16 * hl + 16, tg - 4, :], src)
                    elif r < 32:
                        nc.sync.dma_start(khold[tg][16 * hl:16 * hl + 16, :], src)
                    else:
                        nc.sync.dma_start(vhold[tg][16 * hl:16 * hl + 16, :], src)
                if ct % 3 == 2:
                    tg = 4 + ct // 3
                    psA2 = psum3.tile([128, 136], F32, tag='psA2', name='psA2')
                    for st in range(32):
                        sl = slice(128 * st, 128 * (st + 1))
                        psTk = psum3.tile([128, 128, 2], FP8, tag='trk', bufs=2, name='psTk')
                        nc.tensor.transpose(psTk[:, :, 0], khold[tg][:, sl], id8[:])
                        kTt = trp.tile([128, 128], FP8, tag='kTt', name='kTt')
                        if st % 2 == 0:
                            nc.vector.tensor_scalar_max(kTt[:], psTk[:, :, 0], 0.0)
                        else:
                            nc.scalar.activation(kTt[:], psTk[:, :, 0],
                                                 mybir.ActivationFunctionType.Relu)
                        psTv = psum3.tile([128, 128, 2], FP8, tag='trv', bufs=2, name='psTv')
                        nc.tensor.transpose(psTv[:, :, 0], vhold[tg][:, sl], id8[:])
                        vTt = vTts[st % 4]
                        if st % 2 == 1:
                            nc.vector.tensor_copy(vTt[:, 0:128], psTv[:, :, 0])
                        else:
                            nc.scalar.activation(vTt[:, 0:128], psTv[:, :, 0],
                                                 mybir.ActivationFunctionType.Copy)
                        nc.tensor.matmul(psA2[:, 0:129], kTt[:], vTt[:, 0:129],
                                         start=(st == 0), stop=(st == 31))
                    evac_stage_a(tg, psA2)
                    del khold[tg], vhold[tg]

            if dbg:
                nc.sync.dma_start(dbg_qkv.ap(), qkv[:])
                nc.sync.dma_start(dbg_qstc.ap(), qstc[:])
                nc.sync.dma_start(dbg_qsta.ap(), qsta[:])

            es_p3.close()
            es3.close()   # frees w2c, lt, ast, holds
            es_qkv.close()  # frees padded qkv

            # ---------------- P5+P6 fused, nt-outer ----------------
            wpp = ctx.enter_context(tc.tile_pool(name='wpp', bufs=1))
            wp = wpp.tile([128, 8, 512], BF16)
            nc.sync.dma_start(wp[:], wp_d.ap())
            bnb = wpp.tile([128, 4], F32)
            nc.sync.dma_start(bnb[:], bnb_d.ap())

            psum5 = ctx.enter_context(tc.tile_pool(name='psum5', bufs=1, space='PSUM'))
            qbp = ctx.enter_context(tc.tile_pool(name='qbp', bufs=1))
            attnp = ctx.enter_context(tc.tile_pool(name='attnp', bufs=2))
            drp = ctx.enter_context(tc.tile_pool(name='drp', bufs=2))

            qbs = {}
            for tg in range(8):
                qb = qbp.tile([128, S], BF16, name=f'qb{tg}')
                if tg < 4:
                    nc.vector.tensor_scalar_max(qb[:, 0:2048], qstc[:, tg, 0:2048], 0.0)
                    nc.scalar.activation(qb[:, 2048:4096], qstc[:, tg, 2048:4096],
                                         mybir.ActivationFunctionType.Relu)
                else:
                    nc.vector.tensor_scalar_max(qb[:, 0:2048], qsta[:, tg - 4, 0:2048], 0.0)
                    nc.scalar.activation(qb[:, 2048:4096], qsta[:, tg - 4, 2048:4096],
                                         mybir.ActivationFunctionType.Relu)
                qbs[tg] = qb

            with tc.tile_pool(name='ysp', bufs=3) as ysp:
                for nt in range(NNT):
                    sl = slice(512 * nt, 512 * (nt + 1))
                    attn_nt = attnp.tile([128, 8, 512], BF16, tag='attn_nt', name='attn_nt')
                    for tg in range(8):
                        qb = qbs[tg]
                        csc = 1.0 if tg < 4 else 1.0 / AST_SCALE
                        psN = psum5.tile([128, 512], F32, tag='psN', bufs=2, name='psN')
                        nc.tensor.matmul(psN[:], bdn[:, tg, :], qb[:, sl],
                                         start=True, stop=True)
                        psD = psum5.tile([128, 512], F32, tag='psD', bufs=2, name='psD')
                        nc.tensor.matmul(psD[:], bdd[:, tg, :], qb[:, sl],
                                         start=True, stop=True)
                        drt = drp.tile([128, 512], F32, tag='drt', name='drt')
                        nc.scalar.activation(drt[:], psD[:],
                                             mybir.ActivationFunctionType.Copy, bias=EPS)
                        nc.vector.reciprocal_approx_fast(drt[:], drt[:])
                        nc.vector.scalar_tensor_tensor(
                            attn_nt[:, tg, :], psN[:], csc, drt[:],
                            mybir.AluOpType.mult, mybir.AluOpType.mult)
                    for mt in range(4):
                        ps = psum5.tile([128, 512], F32, tag='psy', bufs=2, name='psy')
                        for kt in range(8):
                            nc.tensor.matmul(ps[:], wp[:, kt, 128 * mt:128 * (mt + 1)],
                                             attn_nt[:, kt, :],
                                             start=(kt == 0), stop=(kt == 7))
                        ys = ysp.tile([128, 512], F32, tag='ys')
                        if mt % 2 == 0:
                            nc.vector.tensor_scalar_add(ys[:], ps[:], bnb[:, mt:mt + 1])
                        else:
                            nc.scalar.activation(ys[:], ps[:],
                                                 mybir.ActivationFunctionType.Identity,
                                                 bias=bnb[:, mt:mt + 1])
                        nc.sync.dma_start(
                            y_d.ap()[128 * mt:128 * (mt + 1), 512 * nt:512 * (nt + 1)], ys[:])

    nc.compile()
    return nc


def host_weights(w_qkv, w_dw, w_pw, w_proj, bn_gamma, bn_beta, bn_mean, bn_var):
    wq = w_qkv[:, :, 0, 0].astype(np.float32)       # [1536, 512]
    wdw = w_dw[:, 0].reshape(1536, 25).astype(np.float32)
    wpw = w_pw[:, :, 0, 0].astype(np.float32)       # [1536, 32]

    # normal qkv weights, lhsT per k-tile
    wq_dev = np.ascontiguousarray(
        wq.T.reshape(NKT, 128, C3).transpose(1, 0, 2)).astype(ml_dtypes.bfloat16)

    # transposed-qkv rhs: k/v channels only, ordered [k: h0..h31 | v: h0..h31]
    kv_cols = np.empty(1024, np.int64)
    for h in range(32):
        kv_cols[16 * h:16 * h + 16] = 48 * h + 16 + np.arange(16)
        kv_cols[512 + 16 * h:512 + 16 * h + 16] = 48 * h + 32 + np.arange(16)
    wqT_dev = np.ascontiguousarray(
        wq[kv_cols].T.reshape(NKT, 128, 1024).transpose(1, 0, 2)).astype(ml_dtypes.bfloat16)

    q_cols = np.empty(512, np.int64)
    for h in range(32):
        q_cols[16 * h:16 * h + 16] = 48 * h + np.arange(16)
    wqq_dev = np.ascontiguousarray(
        wq[q_cols].T.reshape(NKT, 128, 512).transpose(1, 0, 2)).astype(ml_dtypes.bfloat16)

    # fused agg weights (compact block-diag), slot-ordered, scaled 2^14 -> fp8
    A = wdw.reshape(12, 4, 32, 25)
    Bm = wpw.reshape(12, 4, 32, 32)
    W2c = A[:, :, :, :, None] * Bm.transpose(0, 1, 3, 2)[:, :, :, None, :]
    W2c = W2c.transpose(1, 2, 3, 0, 4).reshape(128, 25, 12, 32)   # [row, tap, ct, o]
    slot_idx = [5 * dy + dx for (dy, dx) in SLOT_TAPS]
    W2s = np.clip(W2c[:, slot_idx] * W2_SCALE, -240.0, 240.0)
    w2c_dev = np.ascontiguousarray(W2s).astype(ml_dtypes.float8_e4m3)

    inv = bn_gamma / np.sqrt(bn_var + BN_EPS)
    wp_f = (w_proj[:, :, 0, 0] * inv[:, None]).T    # [1024, 512] lhsT
    bnb = (bn_beta - bn_mean * inv).astype(np.float32)
    wp_dev = np.ascontiguousarray(
        wp_f.reshape(8, 128, 512).transpose(1, 0, 2)).astype(ml_dtypes.bfloat16)
    bnb_dev = np.ascontiguousarray(bnb.reshape(4, 128).T).astype(np.float32)

    obd32 = np.zeros((128, 32), np.float32)
    for r in range(128):
        a = (r % 32) // 16
        obd32[r, 16 * a:16 * a + 16] = 1.0
    obd128 = np.zeros((128, 128), np.float32)
    for r in range(128):
        a = r // 16
        obd128[r, 16 * a:16 * a + 16] = 1.0
    return {'wq': wq_dev, 'wqT': wqT_dev, 'wqq': wqq_dev, 'w2c': w2c_dev, 'wp': wp_dev,
            'bnb': bnb_dev,
            'obd32': obd32.astype(ml_dtypes.bfloat16),
            'obd128': obd128.astype(ml_dtypes.bfloat16)}


def host_x(xb):
    """[512, 4096] f32 -> [128, 4, 4096] bf16 tile layout."""
    return np.ascontiguousarray(
        xb.reshape(NKT, 128, S).transpose(1, 0, 2)).astype(ml_dtypes.bfloat16)


def kernel(x, w_qkv, w_dw, w_pw, w_proj, bn_gamma, bn_beta, bn_mean, bn_var):
    x = np.asarray(x, dtype=np.float32)
    wdev = host_weights(
        np.asarray(w_qkv, np.float32), np.asarray(w_dw, np.float32),
        np.asarray(w_pw, np.float32), np.asarray(w_proj, np.float32),
        np.asarray(bn_gamma, np.float32), np.asarray(bn_beta, np.float32),
        np.asarray(bn_mean, np.float32), np.asarray(bn_var, np.float32))

    if 'nc' not in _CACHED:
        _CACHED['nc'] = build_program()
    nc = _CACHED['nc']

    in_maps = []
    for b in range(B):
        in_maps.append({'x8': host_x(x[b].reshape(CIN, S)), **wdev})
    res = run_bass_kernel_spmd(nc, in_maps, list(range(B)))
    y = np.stack([res.results[b]['y_b'].reshape(512, H, W) for b in range(B)])
    return y.astype(np.float32)


# revision 30
# speedup vs baseline: 1.1011x; 1.0015x over previous
# Trainium2 Bass kernel for nn_LiteMultiscaleAttention (8-core data-parallel over batch).
#
# Per core (one batch):
#   P1 qkv = Wqkv @ x (bf16) -> fp8 padded copy (agg rhs) + bf16 q-rows (stage B)
#   P2 qkvT = xT @ WqkvT (bf16, k/v cols only) streamed into stage-A vk matmuls
#      (kills the 256 PE transposes for clean heads)
#   P3 agg = fused dw5x5+grouped-pw as 25 block-diag taps in fp8, with
#      DoubleRow perf mode pairing vertical taps (2 taps/instr, row stride 80)
#   P4 stage-A for agg heads: fp8 PE transposes + vk matmuls
#   P5 stage-B: 2 matmuls per (tg, nt): block-diag num [128x128], rank-1 den
#   P6 proj + BN (bf16)
import sys
import os
import numpy as np

sys.path.insert(0, '/opt/trn_rl_repo')

import ml_dtypes
import concourse.bass as bass
import concourse.mybir as mybir
import concourse.tile as tile
from concourse import bacc
from concourse.bass_utils import run_bass_kernel_spmd
from concourse.masks import make_identity

BF16 = mybir.dt.bfloat16
F32 = mybir.dt.float32
FP8 = mybir.dt.float8e4
DR = mybir.MatmulPerfMode.DoubleRow

B, CIN, H, W = 8, 512, 64, 64
S = H * W                 # 4096
C3 = 1536
NCT = 12                  # channel tiles of qkv/agg
NKT = 4                   # k-tiles of Cin
NNT = 8                   # 512-wide spatial chunks (8 image rows each)
EPS = 1e-15
BN_EPS = 1e-5
RS = 80                   # padded row stride (so DoubleRow pair step % 16 == 0)
NR = 68                   # padded rows
W2_SCALE = 2.0 ** 14      # host scale on fused agg weights (fp8 range)
AST_SCALE = 2.0 ** 6      # storage scale of agg outputs in fp8
AST_EVAC = AST_SCALE / W2_SCALE

# tap slots: pairs (dy,dy+1) same dx -> DoubleRow; dy=4 row as singles
PAIRS = [((0, dx), (1, dx)) for dx in range(5)] + [((2, dx), (3, dx)) for dx in range(5)]
SINGLES = [(4, dx) for dx in range(5)]
SLOT_TAPS = []
for (a, b_) in PAIRS:
    SLOT_TAPS += [a, b_]
SLOT_TAPS += SINGLES      # 25 slots: [0..19] pair slots, [20..24] singles

_CACHED = {}


def _qkv_unit_role(c):
    """channel c -> (head, role, lane) with role 0=q 1=k 2=v."""
    return c // 48, (c % 48) // 16, c % 16


def build_program(dbg=False):
    nc = bacc.Bacc('TRN2', target_bir_lowering=False, debug=False)

    x8_d = nc.dram_tensor('x8', [128, NKT, S], BF16, kind='ExternalInput')
    wq_d = nc.dram_tensor('wq', [128, NKT, C3], BF16, kind='ExternalInput')
    wqT_d = nc.dram_tensor('wqT', [128, NKT, 1024], BF16, kind='ExternalInput')
    wqq_d = nc.dram_tensor('wqq', [128, NKT, 512], BF16, kind='ExternalInput')
    w2c_d = nc.dram_tensor('w2c', [128, 25, NCT, 32], FP8, kind='ExternalInput')
    wp_d = nc.dram_tensor('wp', [128, 8, 512], BF16, kind='ExternalInput')
    bnb_d = nc.dram_tensor('bnb', [128, 4], F32, kind='ExternalInput')
    obd32_d = nc.dram_tensor('obd32', [128, 32], BF16, kind='ExternalInput')
    obd128_d = nc.dram_tensor('obd128', [128, 128], BF16, kind='ExternalInput')
    y_d = nc.dram_tensor('y_b', [512, S], F32, kind='ExternalOutput')
    if dbg:
        dbg_qkv = nc.dram_tensor('dbg_qkv', [128, NCT, NR, RS], FP8, kind='ExternalOutput')
        dbg_qstc = nc.dram_tensor('dbg_qstc', [128, 4, S], BF16, kind='ExternalOutput')
        dbg_qsta = nc.dram_tensor('dbg_qsta', [128, 4, S], FP8, kind='ExternalOutput')
        dbg_bdn = nc.dram_tensor('dbg_bdn', [128, 8, 128], BF16, kind='ExternalOutput')
        dbg_bdd = nc.dram_tensor('dbg_bdd', [128, 8, 128], BF16, kind='ExternalOutput')
        dbg_den = nc.dram_tensor('dbg_den', [128, 8, 1], F32, kind='ExternalOutput')
        dbg_attn = nc.dram_tensor('dbg_attn', [128, 8, S], BF16, kind='ExternalOutput')
        dbg_kh = nc.dram_tensor('dbg_kh', [128, 4, S], FP8, kind='ExternalOutput')
        dbg_vh = nc.dram_tensor('dbg_vh', [128, 4, S], FP8, kind='ExternalOutput')

    with tile.TileContext(nc) as tc:
        from contextlib import ExitStack
        ctx = ExitStack()
        with ctx:
            stat = ctx.enter_context(tc.tile_pool(name='stat', bufs=1))
            id8 = stat.tile([128, 128], FP8)
            make_identity(nc, id8[:])
            ones_b = stat.tile([128, 1], BF16)
            nc.gpsimd.memset(ones_b[:], 1.0)
            ones_8 = stat.tile([128, 1], FP8)
            nc.gpsimd.memset(ones_8[:], 1.0)
            ones512 = stat.tile([1, 512], BF16)
            nc.gpsimd.memset(ones512[:], 1.0)
            epsw = stat.tile([1, 128], BF16)
            nc.gpsimd.memset(epsw[:], EPS)
            obd32 = stat.tile([128, 32], BF16)
            nc.sync.dma_start(obd32[:], obd32_d.ap())
            obd128 = stat.tile([128, 128], BF16)
            nc.sync.dma_start(obd128[:], obd128_d.ap())

            bdn = stat.tile([128, 8, 128], BF16)
            nc.gpsimd.memset(bdn[:], 0.0)
            bdd = stat.tile([128, 8, 128], BF16)
            nc.gpsimd.memset(bdd[:], 0.0)
            den_col = stat.tile([128, 8, 1], F32)

            # long-lived data tiles
            qstcp = ctx.enter_context(tc.tile_pool(name='qstcp', bufs=1))
            qstc = qstcp.tile([128, 4, S], BF16)      # clean q (heads 0-31)
            qstap = ctx.enter_context(tc.tile_pool(name='qstap', bufs=1))
            qsta = qstap.tile([128, 4, S], FP8)       # agg q (heads 32-63)

            es_qkv = ExitStack()
            qkvp = es_qkv.enter_context(tc.tile_pool(name='qkvp', bufs=1))
            qkv = qkvp.tile([128, NCT, NR, RS], FP8)
            for _ct in range(NCT):
                nc.gpsimd.memset(qkv[:, _ct, 0:2, 0:68], 0.0)
                nc.gpsimd.memset(qkv[:, _ct, 66:68, 0:68], 0.0)
                nc.gpsimd.memset(qkv[:, _ct, 2:66, 0:2], 0.0)
                nc.gpsimd.memset(qkv[:, _ct, 2:66, 66:68], 0.0)

            # ---------------- P0/P1: qkv normal ----------------
            es1 = ExitStack()
            w1 = es1.enter_context(tc.tile_pool(name='w1', bufs=1))
            wq = w1.tile([128, NKT, C3], BF16)
            wqT = w1.tile([128, NKT, 1024], BF16)
            wqq = w1.tile([128, NKT, 512], BF16)
            x8 = w1.tile([128, NKT, S], BF16)
            for kt in range(NKT):
                nc.sync.dma_start(wq[:, kt, :], wq_d.ap()[:, kt, :])
                if kt == 0:
                    for xh in range(4):
                        nc.sync.dma_start(x8[:, 0, 1024 * xh:1024 * (xh + 1)],
                                          x8_d.ap()[:, 0, 1024 * xh:1024 * (xh + 1)])
                else:
                    nc.sync.dma_start(x8[:, kt, :], x8_d.ap()[:, kt, :])
            nc.sync.dma_start(wqT[:], wqT_d.ap())
            nc.sync.dma_start(wqq[:], wqq_d.ap())

            es_p1 = ExitStack()
            psum1 = es_p1.enter_context(tc.tile_pool(name='psum1', bufs=1, space='PSUM'))
            ev = [nc.vector, nc.scalar]
            evi = 0
            for ct in range(NCT):
                for half in range(2):
                    pss = {}
                    for kt in range(NKT):
                        for q4 in range(4):
                            nt = 4 * half + q4
                            if kt == 0:
                                pss[nt] = psum1.tile([128, 512], F32, tag=f'ps{q4}', bufs=2, name=f'p1ps{q4}')
                            nc.tensor.matmul(
                                pss[nt][:], wq[:, kt, 128 * ct:128 * (ct + 1)],
                                x8[:, kt, 512 * nt:512 * (nt + 1)],
                                start=(kt == 0), stop=(kt == NKT - 1))
                    for q4 in range(4):
                        nt = 4 * half + q4
                        ps = pss[nt]
                        dst = qkv[:, ct, 8 * nt + 2:8 * nt + 10, 2:66]
                        if (nt % 2) == 0:
                            nc.vector.tensor_copy(dst, ps[:])
                        else:
                            nc.scalar.activation(dst, ps[:], mybir.ActivationFunctionType.Copy)

            # P1b: q-channels again, head-major -> clean q-stack (full-tile evacs)
            for qt in range(4):
                for half in range(2):
                    pss = {}
                    for kt in range(NKT):
                        for q4 in range(4):
                            nt = 4 * half + q4
                            if kt == 0:
                                pss[nt] = psum1.tile([128, 512], F32, tag=f'ps{q4}', bufs=2, name=f'pbps{q4}')
                            nc.tensor.matmul(
                                pss[nt][:], wqq[:, kt, 128 * qt:128 * (qt + 1)],
                                x8[:, kt, 512 * nt:512 * (nt + 1)],
                                start=(kt == 0), stop=(kt == NKT - 1))
                    for q4 in range(4):
                        nt = 4 * half + q4
                        dq = qstc[:, qt, 512 * nt:512 * (nt + 1)]
                        if (nt % 2) == 0:
                            nc.vector.tensor_scalar_max(dq, pss[nt][:], 0.0)
                        else:
                            nc.scalar.activation(dq, pss[nt][:], mybir.ActivationFunctionType.Relu)

            # ---------------- P2: qkvT + stage A for clean heads ----------------
            es_p1.close()
            es_p2 = ExitStack()
            psum2 = es_p2.enter_context(tc.tile_pool(name='psum2', bufs=1, space='PSUM'))
            psA = {}
            for tg in range(4):
                psA[tg] = psum2.tile([128, 136], F32, tag=f'psA{tg}', bufs=1, name=f'psA{tg}')
            with tc.tile_pool(name='tkv', bufs=2) as tkvp:
                tvs = [tkvp.tile([128, 4, 132], BF16, tag='tv', name=f'tv{i}') for i in range(2)]
                for i in range(2):
                    nc.gpsimd.memset(tvs[i][:, :, 128:129], 1.0)
                for c in range(32):
                    psT = psum2.tile([128, 1024], F32, tag='psT', bufs=2)
                    xa = x8[:, :, 128 * c:128 * (c + 1)]
                    for kt in range(NKT):
                        nc.tensor.matmul(psT[:, 0:512], xa[:, kt, :], wqT[:, kt, 0:512],
                                         start=(kt == 0), stop=(kt == NKT - 1))
                    for kt in range(NKT):
                        nc.tensor.matmul(psT[:, 512:1024], xa[:, kt, :], wqT[:, kt, 512:1024],
                                         start=(kt == 0), stop=(kt == NKT - 1))
                    tk = tkvp.tile([128, 512], BF16, tag='tk')
                    tv = tvs[c % 2]
                    nc.vector.tensor_scalar_max(tk[:], psT[:, 0:512], 0.0)
                    nc.scalar.activation(tv[:, :, 0:128], psT[:, 512:1024],
                                         mybir.ActivationFunctionType.Copy)
                    for tg in range(4):
                        sl = slice(128 * tg, 128 * (tg + 1))
                        nc.tensor.matmul(psA[tg][:, 0:129], tk[:, sl], tv[:, tg, 0:129],
                                         start=(c == 0), stop=(c == 31))

            def evac_stage_a(tg, ps):
                for j in range(4):
                    r = slice(32 * j, 32 * (j + 1))
                    nc.vector.scalar_tensor_tensor(
                        bdn[r, tg, 32 * j:32 * j + 32], ps[r, 32 * j:32 * j + 32], 1.0,
                        obd32[r, :], mybir.AluOpType.mult, mybir.AluOpType.mult)
                nc.vector.tensor_copy(den_col[:, tg, :], ps[:, 128:129])
                nc.vector.tensor_scalar_mul(bdd[:, tg, :], obd128[:], den_col[:, tg, :])

            for tg in range(4):
                evac_stage_a(tg, psA[tg])
            es_p2.close()
            es1.close()   # frees x8/wq/wqT

            # ---------------- P3: agg (fp8 DoubleRow taps) ----------------
            es3 = ExitStack()
            w2cp = es3.enter_context(tc.tile_pool(name='w2cp', bufs=1))
            w2c = w2cp.tile([128, 25, NCT, 32], FP8)
            nc.sync.dma_start(w2c[:], w2c_d.ap())
            holdp = es3.enter_context(tc.tile_pool(name='holdp', bufs=2))
            khold = {}
            vhold = {}
            trp = es3.enter_context(tc.tile_pool(name='trp', bufs=4))
            vTts = [trp.tile([128, 132], FP8, tag='vTt', name=f'vTt{i}') for i in range(4)]
            for i in range(4):
                nc.gpsimd.memset(vTts[i][:, 128:129], 1.0)
            lhsp = es3.enter_context(tc.tile_pool(name='lhsp', bufs=1))
            lts = [lhsp.tile([128, 25, 128], FP8, name=f'lt{i}') for i in range(2)]
            for i in range(2):
                nc.gpsimd.memset(lts[i][:], 0.0)
            astp = es3.enter_context(tc.tile_pool(name='astp', bufs=2))
            es_p3 = ExitStack()
            psum3 = es_p3.enter_context(tc.tile_pool(name='psum3', bufs=1, space='PSUM'))

            def pair_rhs(ct, nt, dy, dx, ddy):
                a = qkv[:, ct, 8 * nt + dy:8 * nt + dy + 8, dx:dx + 64]
                newdims = [list(a.ap[0])] + [[RS * ddy, 2]] + [list(d) for d in a.ap[1:]]
                return bass.AP(a.tensor, a.offset, newdims)

            for ct in range(NCT):
                lt = lts[ct % 2]
                for blk in range(4):
                    nc.sync.dma_start(lt[32 * blk:32 * blk + 32, :, 32 * blk:32 * blk + 32],
                                      w2c[32 * blk:32 * blk + 32, :, ct, :])
                ast = astp.tile([128, S], FP8, tag='ast')
                for nt in range(NNT):
                    ps = psum3.tile([128, 512], F32, tag='ps', bufs=2, name='p3ps')
                    for p, ((dy0, dx0), (dy1, dx1)) in enumerate(PAIRS):
                        nc.tensor.matmul(
                            ps[:], lt[:, 2 * p:2 * p + 2, :],
                            pair_rhs(ct, nt, dy0, dx0, dy1 - dy0),
                            start=(p == 0), stop=False, perf_mode=DR)
                    for si, (dy, dx) in enumerate(SINGLES):
                        nc.tensor.matmul(
                            ps[:], lt[:, 20 + si, :],
                            qkv[:, ct, 8 * nt + dy:8 * nt + dy + 8, dx:dx + 64],
                            start=False, stop=(si == 4))
                    dst = ast[:, 512 * nt:512 * (nt + 1)]
                    if nt % 2 == 0:
                        nc.vector.tensor_scalar_mul(dst, ps[:], AST_EVAC)
                    else:
                        nc.scalar.activation(dst, ps[:],
                                             mybir.ActivationFunctionType.Copy,
                                             scale=AST_EVAC)
                # route 16-row blocks to agg-head destinations
                for bi in range(8):
                    c = 128 * ct + 16 * bi
                    h = 32 + c // 48
                    r = c % 48
                    tg = h // 8
                    hl = h % 8
                    if tg not in khold:
                        khold[tg] = holdp.tile([128, S], FP8, tag='kh', name=f'kh{tg}')
                        vhold[tg] = holdp.tile([128, S], FP8, tag='vh', name=f'vh{tg}')
                    src = ast[16 * bi:16 * bi + 16, :]
                    if r < 16:
                        nc.sync.dma_start(qsta[16 * hl:16 * hl + 16, tg - 4, :], src)
                    elif r < 32:
                        nc.sync.dma_start(khold[tg][16 * hl:16 * hl + 16, :], src)
                    else:
                        nc.sync.dma_start(vhold[tg][16 * hl:16 * hl + 16, :], src)
                if ct % 3 == 2:
                    tg = 4 + ct // 3
                    psA2 = psum3.tile([128, 136], F32, tag='psA2', name='psA2')
                    for st in range(32):
                        sl = slice(128 * st, 128 * (st + 1))
                        psTk = psum3.tile([128, 128, 2], FP8, tag='trk', bufs=2, name='psTk')
                        nc.tensor.transpose(psTk[:, :, 0], khold[tg][:, sl], id8[:])
                        kTt = trp.tile([128, 128], FP8, tag='kTt', name='kTt')
                        if st % 2 == 0:
                            nc.vector.tensor_scalar_max(kTt[:], psTk[:, :, 0], 0.0)
                        else:
                            nc.scalar.activation(kTt[:], psTk[:, :, 0],
                                                 mybir.ActivationFunctionType.Relu)
                        psTv = psum3.tile([128, 128, 2], FP8, tag='trv', bufs=2, name='psTv')
                        nc.tensor.transpose(psTv[:, :, 0], vhold[tg][:, sl], id8[:])
                        vTt = vTts[st % 4]
                        if st % 2 == 1:
                            nc.vector.tensor_copy(vTt[:, 0:128], psTv[:, :, 0])
                        else:
                            nc.scalar.activation(vTt[:, 0:128], psTv[:, :, 0],
                                                 mybir.ActivationFunctionType.Copy)
                        nc.tensor.matmul(psA2[:, 0:129], kTt[:], vTt[:, 0:129],
                                         start=(st == 0), stop=(st == 31))
                    evac_stage_a(tg, psA2)
                    del khold[tg], vhold[tg]

            if dbg:
                nc.sync.dma_start(dbg_qkv.ap(), qkv[:])
                nc.sync.dma_start(dbg_qstc.ap(), qstc[:])
                nc.sync.dma_start(dbg_qsta.ap(), qsta[:])

            es_p3.close()
            es3.close()   # frees w2c, lt, ast, holds
            es_qkv.close()  # frees padded qkv

            # ---------------- P5+P6 fused, nt-outer ----------------
            wpp = ctx.enter_context(tc.tile_pool(name='wpp', bufs=1))
            wp = wpp.tile([128, 8, 512], BF16)
            nc.sync.dma_start(wp[:], wp_d.ap())
            bnb = wpp.tile([128, 4], F32)
            nc.sync.dma_start(bnb[:], bnb_d.ap())

            psum5 = ctx.enter_context(tc.tile_pool(name='psum5', bufs=1, space='PSUM'))
            qbp = ctx.enter_context(tc.tile_pool(name='qbp', bufs=1))
            attnp = ctx.enter_context(tc.tile_pool(name='attnp', bufs=2))
            drp = ctx.enter_context(tc.tile_pool(name='drp', bufs=2))

            qbs = {}
            for tg in range(4):
                qbs[tg] = qstc[:, tg, :]
            for tg in range(4, 8):
                qb = qbp.tile([128, S], BF16, name=f'qb{tg}')
                nc.vector.tensor_scalar_max(qb[:, 0:2048], qsta[:, tg - 4, 0:2048], 0.0)
                nc.scalar.activation(qb[:, 2048:4096], qsta[:, tg - 4, 2048:4096],
                                     mybir.ActivationFunctionType.Relu)
                qbs[tg] = qb

            with tc.tile_pool(name='ysp', bufs=3) as ysp:
                for nt in range(NNT):
                    sl = slice(512 * nt, 512 * (nt + 1))
                    attn_nt = attnp.tile([128, 8, 512], BF16, tag='attn_nt', name='attn_nt')
                    for tg in range(8):
                        qb = qbs[tg]
                        csc = 1.0 if tg < 4 else 1.0 / AST_SCALE
                        qsl = qstc[:, tg, sl] if tg < 4 else qb[:, sl]
                        psN = psum5.tile([128, 512], F32, tag='psN', bufs=2, name='psN')
                        nc.tensor.matmul(psN[:], bdn[:, tg, :], qsl,
                                         start=True, stop=True)
                        psD = psum5.tile([128, 512], F32, tag='psD', bufs=2, name='psD')
                        nc.tensor.matmul(psD[:], bdd[:, tg, :], qsl,
                                         start=True, stop=True)
                        drt = drp.tile([128, 512], F32, tag='drt', name='drt')
                        nc.scalar.activation(drt[:], psD[:],
                                             mybir.ActivationFunctionType.Copy, bias=EPS)
                        nc.vector.reciprocal_approx_fast(drt[:], drt[:])
                        nc.vector.scalar_tensor_tensor(
                            attn_nt[:, tg, :], psN[:], csc, drt[:],
                            mybir.AluOpType.mult, mybir.AluOpType.mult)
                    for mt in range(4):
                        ps = psum5.tile([128, 512], F32, tag='psy', bufs=2, name='psy')
                        for kt in range(8):
                            nc.tensor.matmul(ps[:], wp[:, kt, 128 * mt:128 * (mt + 1)],
                                             attn_nt[:, kt, :],
                                             start=(kt == 0), stop=(kt == 7))
                        ys = ysp.tile([128, 512], F32, tag='ys')
                        if mt % 2 == 0:
                            nc.vector.tensor_scalar_add(ys[:], ps[:], bnb[:, mt:mt + 1])
                        else:
                            nc.scalar.activation(ys[:], ps[:],
                                                 mybir.ActivationFunctionType.Identity,
                                                 bias=bnb[:, mt:mt + 1])
                        nc.sync.dma_start(
                            y_d.ap()[128 * mt:128 * (mt + 1), 512 * nt:512 * (nt + 1)], ys[:])

    nc.compile()
    return nc


def host_weights(w_qkv, w_dw, w_pw, w_proj, bn_gamma, bn_beta, bn_mean, bn_var):
    wq = w_qkv[:, :, 0, 0].astype(np.float32)       # [1536, 512]
    wdw = w_dw[:, 0].reshape(1536, 25).astype(np.float32)
    wpw = w_pw[:, :, 0, 0].astype(np.float32)       # [1536, 32]

    # normal qkv weights, lhsT per k-tile
    wq_dev = np.ascontiguousarray(
        wq.T.reshape(NKT, 128, C3).transpose(1, 0, 2)).astype(ml_dtypes.bfloat16)

    # transposed-qkv rhs: k/v channels only, ordered [k: h0..h31 | v: h0..h31]
    kv_cols = np.empty(1024, np.int64)
    for h in range(32):
        kv_cols[16 * h:16 * h + 16] = 48 * h + 16 + np.arange(16)
        kv_cols[512 + 16 * h:512 + 16 * h + 16] = 48 * h + 32 + np.arange(16)
    wqT_dev = np.ascontiguousarray(
        wq[kv_cols].T.reshape(NKT, 128, 1024).transpose(1, 0, 2)).astype(ml_dtypes.bfloat16)

    q_cols = np.empty(512, np.int64)
    for h in range(32):
        q_cols[16 * h:16 * h + 16] = 48 * h + np.arange(16)
    wqq_dev = np.ascontiguousarray(
        wq[q_cols].T.reshape(NKT, 128, 512).transpose(1, 0, 2)).astype(ml_dtypes.bfloat16)

    # fused agg weights (compact block-diag), slot-ordered, scaled 2^14 -> fp8
    A = wdw.reshape(12, 4, 32, 25)
    Bm = wpw.reshape(12, 4, 32, 32)
    W2c = A[:, :, :, :, None] * Bm.transpose(0, 1, 3, 2)[:, :, :, None, :]
    W2c = W2c.transpose(1, 2, 3, 0, 4).reshape(128, 25, 12, 32)   # [row, tap, ct, o]
    slot_idx = [5 * dy + dx for (dy, dx) in SLOT_TAPS]
    W2s = np.clip(W2c[:, slot_idx] * W2_SCALE, -240.0, 240.0)
    w2c_dev = np.ascontiguousarray(W2s).astype(ml_dtypes.float8_e4m3)

    inv = bn_gamma / np.sqrt(bn_var + BN_EPS)
    wp_f = (w_proj[:, :, 0, 0] * inv[:, None]).T    # [1024, 512] lhsT
    bnb = (bn_beta - bn_mean * inv).astype(np.float32)
    wp_dev = np.ascontiguousarray(
        wp_f.reshape(8, 128, 512).transpose(1, 0, 2)).astype(ml_dtypes.bfloat16)
    bnb_dev = np.ascontiguousarray(bnb.reshape(4, 128).T).astype(np.float32)

    obd32 = np.zeros((128, 32), np.float32)
    for r in range(128):
        a = (r % 32) // 16
        obd32[r, 16 * a:16 * a + 16] = 1.0
    obd128 = np.zeros((128, 128), np.float32)
    for r in range(128):
        a = r // 16
        obd128[r, 16 * a:16 * a + 16] = 1.0
    return {'wq': wq_dev, 'wqT': wqT_dev, 'wqq': wqq_dev, 'w2c': w2c_dev, 'wp': wp_dev,
            'bnb': bnb_dev,
            'obd32': obd32.astype(ml_dtypes.bfloat16),
            'obd128': obd128.astype(ml_dtypes.bfloat16)}


def host_x(xb):
    """[512, 4096] f32 -> [128, 4, 4096] bf16 tile layout."""
    return np.ascontiguousarray(
        xb.reshape(NKT, 128, S).transpose(1, 0, 2)).astype(ml_dtypes.bfloat16)


def kernel(x, w_qkv, w_dw, w_pw, w_proj, bn_gamma, bn_beta, bn_mean, bn_var):
    x = np.asarray(x, dtype=np.float32)
    wdev = host_weights(
        np.asarray(w_qkv, np.float32), np.asarray(w_dw, np.float32),
        np.asarray(w_pw, np.float32), np.asarray(w_proj, np.float32),
        np.asarray(bn_gamma, np.float32), np.asarray(bn_beta, np.float32),
        np.asarray(bn_mean, np.float32), np.asarray(bn_var, np.float32))

    if 'nc' not in _CACHED:
        _CACHED['nc'] = build_program()
    nc = _CACHED['nc']

    in_maps = []
    for b in range(B):
        in_maps.append({'x8': host_x(x[b].reshape(CIN, S)), **wdev})
    res = run_bass_kernel_spmd(nc, in_maps, list(range(B)))
    y = np.stack([res.results[b]['y_b'].reshape(512, H, W) for b in range(B)])
    return y.astype(np.float32)


# revision 31
# speedup vs baseline: 1.1166x; 1.0140x over previous
# Trainium2 Bass kernel for nn_LiteMultiscaleAttention (8-core data-parallel over batch).
#
# Per core (one batch):
#   P1 qkv = Wqkv @ x (bf16) -> fp8 padded copy (agg rhs) + bf16 q-rows (stage B)
#   P2 qkvT = xT @ WqkvT (bf16, k/v cols only) streamed into stage-A vk matmuls
#      (kills the 256 PE transposes for clean heads)
#   P3 agg = fused dw5x5+grouped-pw as 25 block-diag taps in fp8, with
#      DoubleRow perf mode pairing vertical taps (2 taps/instr, row stride 80)
#   P4 stage-A for agg heads: fp8 PE transposes + vk matmuls
#   P5 stage-B: 2 matmuls per (tg, nt): block-diag num [128x128], rank-1 den
#   P6 proj + BN (bf16)
import sys
import os
import numpy as np

sys.path.insert(0, '/opt/trn_rl_repo')

import ml_dtypes
import concourse.bass as bass
import concourse.mybir as mybir
import concourse.tile as tile
from concourse import bacc
from concourse.bass_utils import run_bass_kernel_spmd
from concourse.masks import make_identity

BF16 = mybir.dt.bfloat16
F32 = mybir.dt.float32
FP8 = mybir.dt.float8e4
DR = mybir.MatmulPerfMode.DoubleRow

B, CIN, H, W = 8, 512, 64, 64
S = H * W                 # 4096
C3 = 1536
NCT = 12                  # channel tiles of qkv/agg
NKT = 4                   # k-tiles of Cin
NNT = 8                   # 512-wide spatial chunks (8 image rows each)
EPS = 1e-15
BN_EPS = 1e-5
RS = 80                   # padded row stride (so DoubleRow pair step % 16 == 0)
NR = 68                   # padded rows
W2_SCALE = 2.0 ** 14      # host scale on fused agg weights (fp8 range)
AST_SCALE = 2.0 ** 6      # storage scale of agg outputs in fp8
AST_EVAC = AST_SCALE / W2_SCALE

# tap slots: pairs (dy,dy+1) same dx -> DoubleRow; dy=4 row as singles
PAIRS = [((0, dx), (1, dx)) for dx in range(5)] + [((2, dx), (3, dx)) for dx in range(5)]
SINGLES = [(4, dx) for dx in range(5)]
SLOT_TAPS = []
for (a, b_) in PAIRS:
    SLOT_TAPS += [a, b_]
SLOT_TAPS += SINGLES      # 25 slots: [0..19] pair slots, [20..24] singles

_CACHED = {}


def _qkv_unit_role(c):
    """channel c -> (head, role, lane) with role 0=q 1=k 2=v."""
    return c // 48, (c % 48) // 16, c % 16


def build_program(dbg=False):
    nc = bacc.Bacc('TRN2', target_bir_lowering=False, debug=False)

    x8_d = nc.dram_tensor('x8', [128, NKT, S], BF16, kind='ExternalInput')
    wq_d = nc.dram_tensor('wq', [128, NKT, C3], BF16, kind='ExternalInput')
    wqT_d = nc.dram_tensor('wqT', [128, NKT, 1024], BF16, kind='ExternalInput')
    wqq_d = nc.dram_tensor('wqq', [128, NKT, 512], BF16, kind='ExternalInput')
    w2c_d = nc.dram_tensor('w2c', [128, 25, NCT, 32], FP8, kind='ExternalInput')
    wp_d = nc.dram_tensor('wp', [128, 8, 512], BF16, kind='ExternalInput')
    bnb_d = nc.dram_tensor('bnb', [128, 4], F32, kind='ExternalInput')
    obd32_d = nc.dram_tensor('obd32', [128, 32], BF16, kind='ExternalInput')
    obd128_d = nc.dram_tensor('obd128', [128, 128], BF16, kind='ExternalInput')
    y_d = nc.dram_tensor('y_b', [512, S], F32, kind='ExternalOutput')
    if dbg:
        dbg_qkv = nc.dram_tensor('dbg_qkv', [128, NCT, NR, RS], FP8, kind='ExternalOutput')
        dbg_qstc = nc.dram_tensor('dbg_qstc', [128, 4, S], BF16, kind='ExternalOutput')
        dbg_qsta = nc.dram_tensor('dbg_qsta', [128, 4, S], FP8, kind='ExternalOutput')
        dbg_bdn = nc.dram_tensor('dbg_bdn', [128, 8, 128], BF16, kind='ExternalOutput')
        dbg_bdd = nc.dram_tensor('dbg_bdd', [128, 8, 128], BF16, kind='ExternalOutput')
        dbg_den = nc.dram_tensor('dbg_den', [128, 8, 1], F32, kind='ExternalOutput')
        dbg_attn = nc.dram_tensor('dbg_attn', [128, 8, S], BF16, kind='ExternalOutput')
        dbg_kh = nc.dram_tensor('dbg_kh', [128, 4, S], FP8, kind='ExternalOutput')
        dbg_vh = nc.dram_tensor('dbg_vh', [128, 4, S], FP8, kind='ExternalOutput')

    with tile.TileContext(nc) as tc:
        from contextlib import ExitStack
        ctx = ExitStack()
        with ctx:
            stat = ctx.enter_context(tc.tile_pool(name='stat', bufs=1))
            id8 = stat.tile([128, 128], FP8)
            make_identity(nc, id8[:])
            ones_b = stat.tile([128, 1], BF16)
            nc.gpsimd.memset(ones_b[:], 1.0)
            ones_8 = stat.tile([128, 1], FP8)
            nc.gpsimd.memset(ones_8[:], 1.0)
            ones512 = stat.tile([1, 512], BF16)
            nc.gpsimd.memset(ones512[:], 1.0)
            epsw = stat.tile([1, 128], BF16)
            nc.gpsimd.memset(epsw[:], EPS)
            obd32 = stat.tile([128, 32], BF16)
            obd128 = stat.tile([128, 128], BF16)

            bdn = stat.tile([128, 8, 128], BF16)
            nc.gpsimd.memset(bdn[:], 0.0)
            bdd = stat.tile([128, 8, 128], BF16)
            nc.gpsimd.memset(bdd[:], 0.0)
            den_col = stat.tile([128, 8, 1], F32)

            # long-lived data tiles
            qstcp = ctx.enter_context(tc.tile_pool(name='qstcp', bufs=1))
            qstc = qstcp.tile([128, 4, S], BF16)      # clean q (heads 0-31)
            qstap = ctx.enter_context(tc.tile_pool(name='qstap', bufs=1))
            qsta = qstap.tile([128, 4, S], FP8)       # agg q (heads 32-63)
            wpp = ctx.enter_context(tc.tile_pool(name='wpp', bufs=1))
            wp = wpp.tile([128, 8, 512], BF16)
            bnb = wpp.tile([128, 4], F32)

            es_qkv = ExitStack()
            qkvp = es_qkv.enter_context(tc.tile_pool(name='qkvp', bufs=1))
            qkv = qkvp.tile([128, NCT, NR, RS], FP8)
            for _ct in range(NCT):
                nc.gpsimd.memset(qkv[:, _ct, 0:2, 0:68], 0.0)
                nc.gpsimd.memset(qkv[:, _ct, 66:68, 0:68], 0.0)
                nc.gpsimd.memset(qkv[:, _ct, 2:66, 0:2], 0.0)
                nc.gpsimd.memset(qkv[:, _ct, 2:66, 66:68], 0.0)

            # ---------------- P0/P1: qkv normal ----------------
            es1 = ExitStack()
            w1 = es1.enter_context(tc.tile_pool(name='w1', bufs=1))
            wq = w1.tile([128, NKT, C3], BF16)
            wqT = w1.tile([128, NKT, 1024], BF16)
            wqq = w1.tile([128, NKT, 512], BF16)
            x8 = w1.tile([128, NKT, S], BF16)
            for kt in range(NKT):
                nc.sync.dma_start(wq[:, kt, :], wq_d.ap()[:, kt, :])
                if kt == 0:
                    for xh in range(4):
                        nc.sync.dma_start(x8[:, 0, 1024 * xh:1024 * (xh + 1)],
                                          x8_d.ap()[:, 0, 1024 * xh:1024 * (xh + 1)])
                else:
                    nc.sync.dma_start(x8[:, kt, :], x8_d.ap()[:, kt, :])
            nc.sync.dma_start(wqT[:], wqT_d.ap())
            nc.sync.dma_start(wqq[:], wqq_d.ap())
            nc.sync.dma_start(obd32[:], obd32_d.ap())
            nc.sync.dma_start(obd128[:], obd128_d.ap())
            nc.sync.dma_start(wp[:], wp_d.ap())
            nc.sync.dma_start(bnb[:], bnb_d.ap())

            es_p1 = ExitStack()
            psum1 = es_p1.enter_context(tc.tile_pool(name='psum1', bufs=1, space='PSUM'))
            ev = [nc.vector, nc.scalar]
            evi = 0
            for ct in range(NCT):
                for half in range(2):
                    pss = {}
                    for kt in range(NKT):
                        for q4 in range(4):
                            nt = 4 * half + q4
                            if kt == 0:
                                pss[nt] = psum1.tile([128, 512], F32, tag=f'ps{q4}', bufs=2, name=f'p1ps{q4}')
                            nc.tensor.matmul(
                                pss[nt][:], wq[:, kt, 128 * ct:128 * (ct + 1)],
                                x8[:, kt, 512 * nt:512 * (nt + 1)],
                                start=(kt == 0), stop=(kt == NKT - 1))
                    for q4 in range(4):
                        nt = 4 * half + q4
                        ps = pss[nt]
                        dst = qkv[:, ct, 8 * nt + 2:8 * nt + 10, 2:66]
                        if (nt % 2) == 0:
                            nc.vector.tensor_copy(dst, ps[:])
                        else:
                            nc.scalar.activation(dst, ps[:], mybir.ActivationFunctionType.Copy)

            # P1b: q-channels again, head-major -> clean q-stack (full-tile evacs)
            for qt in range(4):
                for half in range(2):
                    pss = {}
                    for kt in range(NKT):
                        for q4 in range(4):
                            nt = 4 * half + q4
                            if kt == 0:
                                pss[nt] = psum1.tile([128, 512], F32, tag=f'ps{q4}', bufs=2, name=f'pbps{q4}')
                            nc.tensor.matmul(
                                pss[nt][:], wqq[:, kt, 128 * qt:128 * (qt + 1)],
                                x8[:, kt, 512 * nt:512 * (nt + 1)],
                                start=(kt == 0), stop=(kt == NKT - 1))
                    for q4 in range(4):
                        nt = 4 * half + q4
                        dq = qstc[:, qt, 512 * nt:512 * (nt + 1)]
                        if (nt % 2) == 0:
                            nc.vector.tensor_scalar_max(dq, pss[nt][:], 0.0)
                        else:
                            nc.scalar.activation(dq, pss[nt][:], mybir.ActivationFunctionType.Relu)

            # ---------------- P2: qkvT + stage A for clean heads ----------------
            es_p1.close()
            es_p2 = ExitStack()
            psum2 = es_p2.enter_context(tc.tile_pool(name='psum2', bufs=1, space='PSUM'))
            psA = {}
            for tg in range(4):
                psA[tg] = psum2.tile([128, 136], F32, tag=f'psA{tg}', bufs=1, name=f'psA{tg}')
            with tc.tile_pool(name='tkv', bufs=2) as tkvp:
                tvs = [tkvp.tile([128, 4, 132], BF16, tag='tv', name=f'tv{i}') for i in range(2)]
                for i in range(2):
                    nc.gpsimd.memset(tvs[i][:, :, 128:129], 1.0)
                for c in range(32):
                    psT = psum2.tile([128, 1024], F32, tag='psT', bufs=2)
                    xa = x8[:, :, 128 * c:128 * (c + 1)]
                    for kt in range(NKT):
                        nc.tensor.matmul(psT[:, 0:512], xa[:, kt, :], wqT[:, kt, 0:512],
                                         start=(kt == 0), stop=(kt == NKT - 1))
                    for kt in range(NKT):
                        nc.tensor.matmul(psT[:, 512:1024], xa[:, kt, :], wqT[:, kt, 512:1024],
                                         start=(kt == 0), stop=(kt == NKT - 1))
                    tk = tkvp.tile([128, 512], BF16, tag='tk')
                    tv = tvs[c % 2]
                    nc.vector.tensor_scalar_max(tk[:], psT[:, 0:512], 0.0)
                    nc.scalar.activation(tv[:, :, 0:128], psT[:, 512:1024],
                                         mybir.ActivationFunctionType.Copy)
                    for tg in range(4):
                        sl = slice(128 * tg, 128 * (tg + 1))
                        nc.tensor.matmul(psA[tg][:, 0:129], tk[:, sl], tv[:, tg, 0:129],
                                         start=(c == 0), stop=(c == 31))

            def evac_stage_a(tg, ps):
                for j in range(4):
                    r = slice(32 * j, 32 * (j + 1))
                    nc.vector.scalar_tensor_tensor(
                        bdn[r, tg, 32 * j:32 * j + 32], ps[r, 32 * j:32 * j + 32], 1.0,
                        obd32[r, :], mybir.AluOpType.mult, mybir.AluOpType.mult)
                nc.vector.tensor_copy(den_col[:, tg, :], ps[:, 128:129])
                nc.vector.tensor_scalar_mul(bdd[:, tg, :], obd128[:], den_col[:, tg, :])

            for tg in range(4):
                evac_stage_a(tg, psA[tg])
            es_p2.close()
            es1.close()   # frees x8/wq/wqT

            # ---------------- P3: agg (fp8 DoubleRow taps) ----------------
            es3 = ExitStack()
            w2cp = es3.enter_context(tc.tile_pool(name='w2cp', bufs=1))
            w2c = w2cp.tile([128, 25, NCT, 32], FP8)
            nc.sync.dma_start(w2c[:], w2c_d.ap())
            holdp = es3.enter_context(tc.tile_pool(name='holdp', bufs=2))
            khold = {}
            vhold = {}
            trp = es3.enter_context(tc.tile_pool(name='trp', bufs=4))
            vTts = [trp.tile([128, 132], FP8, tag='vTt', name=f'vTt{i}') for i in range(4)]
            for i in range(4):
                nc.gpsimd.memset(vTts[i][:, 128:129], 1.0)
            lhsp = es3.enter_context(tc.tile_pool(name='lhsp', bufs=1))
            lts = [lhsp.tile([128, 25, 128], FP8, name=f'lt{i}') for i in range(2)]
            for i in range(2):
                nc.gpsimd.memset(lts[i][:], 0.0)
            astp = es3.enter_context(tc.tile_pool(name='astp', bufs=2))
            es_p3 = ExitStack()
            psum3 = es_p3.enter_context(tc.tile_pool(name='psum3', bufs=1, space='PSUM'))

            def pair_rhs(ct, nt, dy, dx, ddy):
                a = qkv[:, ct, 8 * nt + dy:8 * nt + dy + 8, dx:dx + 64]
                newdims = [list(a.ap[0])] + [[RS * ddy, 2]] + [list(d) for d in a.ap[1:]]
                return bass.AP(a.tensor, a.offset, newdims)

            for ct in range(NCT):
                lt = lts[ct % 2]
                for blk in range(4):
                    nc.sync.dma_start(lt[32 * blk:32 * blk + 32, :, 32 * blk:32 * blk + 32],
                                      w2c[32 * blk:32 * blk + 32, :, ct, :])
                ast = astp.tile([128, S], FP8, tag='ast')
                for nt in range(NNT):
                    ps = psum3.tile([128, 512], F32, tag='ps', bufs=2, name='p3ps')
                    for p, ((dy0, dx0), (dy1, dx1)) in enumerate(PAIRS):
                        nc.tensor.matmul(
                            ps[:], lt[:, 2 * p:2 * p + 2, :],
                            pair_rhs(ct, nt, dy0, dx0, dy1 - dy0),
                            start=(p == 0), stop=False, perf_mode=DR)
                    for si, (dy, dx) in enumerate(SINGLES):
                        nc.tensor.matmul(
                            ps[:], lt[:, 20 + si, :],
                            qkv[:, ct, 8 * nt + dy:8 * nt + dy + 8, dx:dx + 64],
                            start=False, stop=(si == 4))
                    dst = ast[:, 512 * nt:512 * (nt + 1)]
                    if nt % 2 == 0:
                        nc.vector.tensor_scalar_mul(dst, ps[:], AST_EVAC)
                    else:
                        nc.scalar.activation(dst, ps[:],
                                             mybir.ActivationFunctionType.Copy,
                                             scale=AST_EVAC)
                # route 16-row blocks to agg-head destinations
                for bi in range(8):
                    c = 128 * ct + 16 * bi
                    h = 32 + c // 48
                    r = c % 48
                    tg = h // 8
                    hl = h % 8
                    if tg not in khold:
                        khold[tg] = holdp.tile([128, S], FP8, tag='kh', name=f'kh{tg}')
                        vhold[tg] = holdp.tile([128, S], FP8, tag='vh', name=f'vh{tg}')
                    src = ast[16 * bi:16 * bi + 16, :]
                    if r < 16:
                        nc.sync.dma_start(qsta[16 * hl:16 * hl + 16, tg - 4, :], src)
                    elif r < 32:
                        nc.sync.dma_start(khold[tg][16 * hl:16 * hl + 16, :], src)
                    else:
                        nc.sync.dma_start(vhold[tg][16 * hl:16 * hl + 16, :], src)
                if ct % 3 == 2:
                    tg = 4 + ct // 3
                    psA2 = psum3.tile([128, 136], F32, tag='psA2', name='psA2')
                    for st in range(32):
                        sl = slice(128 * st, 128 * (st + 1))
                        psTk = psum3.tile([128, 128, 2], FP8, tag='trk', bufs=2, name='psTk')
                        nc.tensor.transpose(psTk[:, :, 0], khold[tg][:, sl], id8[:])
                        kTt = trp.tile([128, 128], FP8, tag='kTt', name='kTt')
                        if st % 2 == 0:
                            nc.vector.tensor_scalar_max(kTt[:], psTk[:, :, 0], 0.0)
                        else:
                            nc.scalar.activation(kTt[:], psTk[:, :, 0],
                                                 mybir.ActivationFunctionType.Relu)
                        psTv = psum3.tile([128, 128, 2], FP8, tag='trv', bufs=2, name='psTv')
                        nc.tensor.transpose(psTv[:, :, 0], vhold[tg][:, sl], id8[:])
                        vTt = vTts[st % 4]
                        if st % 2 == 1:
                            nc.vector.tensor_copy(vTt[:, 0:128], psTv[:, :, 0])
                        else:
                            nc.scalar.activation(vTt[:, 0:128], psTv[:, :, 0],
                                                 mybir.ActivationFunctionType.Copy)
                        nc.tensor.matmul(psA2[:, 0:129], kTt[:], vTt[:, 0:129],
                                         start=(st == 0), stop=(st == 31))
                    evac_stage_a(tg, psA2)
                    del khold[tg], vhold[tg]

            if dbg:
                nc.sync.dma_start(dbg_qkv.ap(), qkv[:])
                nc.sync.dma_start(dbg_qstc.ap(), qstc[:])
                nc.sync.dma_start(dbg_qsta.ap(), qsta[:])

            es_p3.close()
            es3.close()   # frees w2c, lt, ast, holds
            es_qkv.close()  # frees padded qkv

            # ---------------- P5+P6 fused, nt-outer ----------------
            psum5 = ctx.enter_context(tc.tile_pool(name='psum5', bufs=1, space='PSUM'))
            qbp = ctx.enter_context(tc.tile_pool(name='qbp', bufs=1))
            attnp = ctx.enter_context(tc.tile_pool(name='attnp', bufs=2))
            drp = ctx.enter_context(tc.tile_pool(name='drp', bufs=2))

            qbs = {}
            for tg in range(4):
                qbs[tg] = qstc[:, tg, :]
            for tg in range(4, 8):
                qb = qbp.tile([128, S], BF16, name=f'qb{tg}')
                nc.vector.tensor_scalar_max(qb[:, 0:2048], qsta[:, tg - 4, 0:2048], 0.0)
                nc.scalar.activation(qb[:, 2048:4096], qsta[:, tg - 4, 2048:4096],
                                     mybir.ActivationFunctionType.Relu)
                qbs[tg] = qb

            with tc.tile_pool(name='ysp', bufs=3) as ysp:
                for nt in range(NNT):
                    sl = slice(512 * nt, 512 * (nt + 1))
                    attn_nt = attnp.tile([128, 8, 512], BF16, tag='attn_nt', name='attn_nt')
                    for tg in range(8):
                        qb = qbs[tg]
                        csc = 1.0 if tg < 4 else 1.0 / AST_SCALE
                        qsl = qstc[:, tg, sl] if tg < 4 else qb[:, sl]
                        psN = psum5.tile([128, 512], F32, tag='psN', bufs=2, name='psN')
                        nc.tensor.matmul(psN[:], bdn[:, tg, :], qsl,
                                         start=True, stop=True)
                        psD = psum5.tile([128, 512], F32, tag='psD', bufs=2, name='psD')
                        nc.tensor.matmul(psD[:], bdd[:, tg, :], qsl,
                                         start=True, stop=True)
                        drt = drp.tile([128, 512], F32, tag='drt', name='drt')
                        nc.scalar.activation(drt[:], psD[:],
                                             mybir.ActivationFunctionType.Copy, bias=EPS)
                        nc.vector.reciprocal_approx_fast(drt[:], drt[:])
                        nc.vector.scalar_tensor_tensor(
                            attn_nt[:, tg, :], psN[:], csc, drt[:],
                            mybir.AluOpType.mult, mybir.AluOpType.mult)
                    for mt in range(4):
                        ps = psum5.tile([128, 512], F32, tag='psy', bufs=2, name='psy')
                        for kt in range(8):
                            nc.tensor.matmul(ps[:], wp[:, kt, 128 * mt:128 * (mt + 1)],
                                             attn_nt[:, kt, :],
                                             start=(kt == 0), stop=(kt == 7))
                        ys = ysp.tile([128, 512], F32, tag='ys')
                        if mt % 2 == 0:
                            nc.vector.tensor_scalar_add(ys[:], ps[:], bnb[:, mt:mt + 1])
                        else:
                            nc.scalar.activation(ys[:], ps[:],
                                                 mybir.ActivationFunctionType.Identity,
                                                 bias=bnb[:, mt:mt + 1])
                        nc.sync.dma_start(
                            y_d.ap()[128 * mt:128 * (mt + 1), 512 * nt:512 * (nt + 1)], ys[:])

    nc.compile()
    return nc


def host_weights(w_qkv, w_dw, w_pw, w_proj, bn_gamma, bn_beta, bn_mean, bn_var):
    wq = w_qkv[:, :, 0, 0].astype(np.float32)       # [1536, 512]
    wdw = w_dw[:, 0].reshape(1536, 25).astype(np.float32)
    wpw = w_pw[:, :, 0, 0].astype(np.float32)       # [1536, 32]

    # normal qkv weights, lhsT per k-tile
    wq_dev = np.ascontiguousarray(
        wq.T.reshape(NKT, 128, C3).transpose(1, 0, 2)).astype(ml_dtypes.bfloat16)

    # transposed-qkv rhs: k/v channels only, ordered [k: h0..h31 | v: h0..h31]
    kv_cols = np.empty(1024, np.int64)
    for h in range(32):
        kv_cols[16 * h:16 * h + 16] = 48 * h + 16 + np.arange(16)
        kv_cols[512 + 16 * h:512 + 16 * h + 16] = 48 * h + 32 + np.arange(16)
    wqT_dev = np.ascontiguousarray(
        wq[kv_cols].T.reshape(NKT, 128, 1024).transpose(1, 0, 2)).astype(ml_dtypes.bfloat16)

    q_cols = np.empty(512, np.int64)
    for h in range(32):
        q_cols[16 * h:16 * h + 16] = 48 * h + np.arange(16)
    wqq_dev = np.ascontiguousarray(
        wq[q_cols].T.reshape(NKT, 128, 512).transpose(1, 0, 2)).astype(ml_dtypes.bfloat16)

    # fused agg weights (compact block-diag), slot-ordered, scaled 2^14 -> fp8
    A = wdw.reshape(12, 4, 32, 25)
    Bm = wpw.reshape(12, 4, 32, 32)
    W2c = A[:, :, :, :, None] * Bm.transpose(0, 1, 3, 2)[:, :, :, None, :]
    W2c = W2c.transpose(1, 2, 3, 0, 4).reshape(128, 25, 12, 32)   # [row, tap, ct, o]
    slot_idx = [5 * dy + dx for (dy, dx) in SLOT_TAPS]
    W2s = np.clip(W2c[:, slot_idx] * W2_SCALE, -240.0, 240.0)
    w2c_dev = np.ascontiguousarray(W2s).astype(ml_dtypes.float8_e4m3)

    inv = bn_gamma / np.sqrt(bn_var + BN_EPS)
    wp_f = (w_proj[:, :, 0, 0] * inv[:, None]).T    # [1024, 512] lhsT
    bnb = (bn_beta - bn_mean * inv).astype(np.float32)
    wp_dev = np.ascontiguousarray(
        wp_f.reshape(8, 128, 512).transpose(1, 0, 2)).astype(ml_dtypes.bfloat16)
    bnb_dev = np.ascontiguousarray(bnb.reshape(4, 128).T).astype(np.float32)

    obd32 = np.zeros((128, 32), np.float32)
    for r in range(128):
        a = (r % 32) // 16
        obd32[r, 16 * a:16 * a + 16] = 1.0
    obd128 = np.zeros((128, 128), np.float32)
    for r in range(128):
        a = r // 16
        obd128[r, 16 * a:16 * a + 16] = 1.0
    return {'wq': wq_dev, 'wqT': wqT_dev, 'wqq': wqq_dev, 'w2c': w2c_dev, 'wp': wp_dev,
            'bnb': bnb_dev,
            'obd32': obd32.astype(ml_dtypes.bfloat16),
            'obd128': obd128.astype(ml_dtypes.bfloat16)}


def host_x(xb):
    """[512, 4096] f32 -> [128, 4, 4096] bf16 tile layout."""
    return np.ascontiguousarray(
        xb.reshape(NKT, 128, S).transpose(1, 0, 2)).astype(ml_dtypes.bfloat16)


def kernel(x, w_qkv, w_dw, w_pw, w_proj, bn_gamma, bn_beta, bn_mean, bn_var):
    x = np.asarray(x, dtype=np.float32)
    wdev = host_weights(
        np.asarray(w_qkv, np.float32), np.asarray(w_dw, np.float32),
        np.asarray(w_pw, np.float32), np.asarray(w_proj, np.float32),
        np.asarray(bn_gamma, np.float32), np.asarray(bn_beta, np.float32),
        np.asarray(bn_mean, np.float32), np.asarray(bn_var, np.float32))

    if 'nc' not in _CACHED:
        _CACHED['nc'] = build_program()
    nc = _CACHED['nc']

    in_maps = []
    for b in range(B):
        in_maps.append({'x8': host_x(x[b].reshape(CIN, S)), **wdev})
    res = run_bass_kernel_spmd(nc, in_maps, list(range(B)))
    y = np.stack([res.results[b]['y_b'].reshape(512, H, W) for b in range(B)])
    return y.astype(np.float32)
